# revision 33
# baseline (speedup 1.0000x reference)
"""Trainium2 Bass kernel for DeformableConvBlock.

Reference computation:
    inp    = concat([lr, hr], axis=1)              # [B, 256, 96, 96]
    offset = conv3x3(inp, w_off, b_off)            # [B, 18, 96, 96]
    out    = deform_conv3x3(hr, offset, w_def, b_def)

Sharding: 8 cores = 4 batches x 2 half-images (48 output rows each).
Each core:
  - computes its offset conv slice (18 matmuls / tile, shifted-window APs)
  - computes sample coords/weights on DVE (pixel-partition layout,
    magic-number floor; clamp to [-1+eps, 96] + zero border reproduces
    torchvision OOB semantics exactly)
  - bilinear sampling via ONE dma_gather per tap from a host-built
    y-pair transposed padded fp16 image (each 1KB element holds all
    4 bilinear neighbors: rows (y0,x0),(y0+1,x0),(y0,x0+1),(y0+1,x0+1))
  - weighted 4-neighbor combine on DVE via tensor_scalar +
    scalar_tensor_tensor with per-partition (per-pixel) weight scalars
  - PE-transposes combined samples to channel-partition, einsum as 9
    PSUM-accumulated matmuls per 1152-pixel supertile
"""

import os
import numpy as np
from contextlib import ExitStack

import concourse.bass as bass
import concourse.tile as tile
from concourse import bacc, mybir
from concourse.bass_utils import run_bass_kernel_spmd

F32 = mybir.dt.float32
FP16 = mybir.dt.float16
I16 = mybir.dt.int16
ALU = mybir.AluOpType

# ---- geometry (hardcoded for the 4x128x96x96 problem) ----
B, C, H, W = 4, 128, 96, 96
WP = H + 2                    # 98  padded width
NPIMG = WP * WP               # 9604 padded pixels
NPAD = 76 * 128               # 9728 imgT2 rows (covers 9604 + slack)
HLF = H // 2                  # 48 output rows per core
NQ = HLF * W                  # 4608 output pixels per core
NT = NQ // 128                # 36 pixel tiles
NST = 4                       # supertiles
STT = NT // NST               # 9 tiles per supertile
STQ = STT * 128               # 1152 pixels per supertile
SLAB_ROWS = 50                # conv input rows per core (48 + halo)
SLAB_OFF = WP                 # leading zero row in conv input buffers
SLAB_ALLOC = SLAB_OFF + SLAB_ROWS * WP + 220   # slack for window overread
MAGIC = 12582912.0            # 1.5 * 2**23
CLAMP_LO = -0.9999999
CLAMP_HI = 96.0
NK = 9                        # taps
CONV_NT = 10                  # offset-conv output tiles (5 rows each)
CONV_N = 5 * WP               # 490


def _build_program(num_devices=8, enable_asserts=True, stage=99,
                   multi_queue=True):
    nc = bacc.Bacc(
        "TRN2",
        target_bir_lowering=False,
        debug=False,
        enable_asserts=enable_asserts,
        num_devices=num_devices,
        num_swdge_queues=4 if multi_queue else 1,
    )

    def inp(name, shape, dt):
        return nc.dram_tensor(name, shape, dt, kind="ExternalInput")

    imgT2_t = inp("imgT2", [NPAD, 2 * C], FP16)
    lr_slab_t = inp("lr_slab", [C, SLAB_ROWS * WP], FP16)
    hr_slab_t = inp("hr_slab", [C, SLAB_ROWS * WP], FP16)
    w_off_t = inp("w_off_p", [C, 2 * NK * 18], FP16)
    w_def_t = inp("w_def_p", [C, NK * C], FP16)
    b_off_t = inp("b_off_p", [18, 1], F32)
    b_def_t = inp("b_def_p", [C, 1], F32)
    base_t = inp("base_p", [C, NT, 18], F32)
    id16_t = inp("ident16", [128, 128], FP16)
    id32_t = inp("ident32", [128, 128], F32)

    out_t = nc.dram_tensor("out", [C, NQ], F32, kind="ExternalOutput")

    with tile.TileContext(nc) as tc, ExitStack() as ctx:
        persist = ctx.enter_context(tc.tile_pool(name="persist", bufs=1))

        # ---------------- persistent SBUF tensors ----------------
        inp_lr = persist.tile([C, SLAB_ALLOC], FP16, tag="inp_lr")
        inp_hr = persist.tile([C, SLAB_ALLOC], FP16, tag="inp_hr")
        lhsT_off = persist.tile([C, 2 * NK * 18], FP16, tag="lhsT_off")
        lhsT_def = persist.tile([C, NK * C], FP16, tag="lhsT_def")
        b_off_sb = persist.tile([18, 1], F32, tag="b_off_sb")
        b_def_sb = persist.tile([C, 1], F32, tag="b_def_sb")
        base_sb = persist.tile([C, NT * 18], F32, tag="base_sb")
        id16 = persist.tile([128, 128], FP16, tag="id16")
        id32 = persist.tile([128, 128], F32, tag="id32")
        off_sb = persist.tile([18, NQ], F32, tag="off_sb")
        offT_sb = persist.tile([C, NT * 18], F32, tag="offT_sb")
        # rep-2 weight planes: each weight stored twice contiguously so the
        # broadcast view's innermost dim is step-1 packed pairs (DVE 2x mode)
        w00 = persist.tile([C, NT * NK * 2], FP16, tag="w00")
        w01 = persist.tile([C, NT * NK * 2], FP16, tag="w01")
        w10 = persist.tile([C, NT * NK * 2], FP16, tag="w10")
        w11 = persist.tile([C, NT * NK * 2], FP16, tag="w11")
        idxw = persist.tile([128, NK * NST * (STQ // 16)], I16, tag="idxw")
        idxm = persist.tile([16, 8 * NT * NK], I16, tag="idxm")

        # ---------------- load constants / weights / slabs ----------------
        nc.sync.dma_start(out=lhsT_off[:], in_=w_off_t[:])
        nc.sync.dma_start(out=lhsT_def[:], in_=w_def_t[:])
        nc.sync.dma_start(out=b_off_sb[:], in_=b_off_t[:])
        nc.sync.dma_start(out=b_def_sb[:], in_=b_def_t[:])
        nc.sync.dma_start(out=base_sb[:], in_=base_t[:].rearrange("p t c -> p (t c)"))
        nc.sync.dma_start(out=id16[:], in_=id16_t[:])
        nc.sync.dma_start(out=id32[:], in_=id32_t[:])

        nc.vector.memset(inp_lr[:, 0:SLAB_OFF], 0.0)
        nc.vector.memset(inp_lr[:, SLAB_OFF + SLAB_ROWS * WP:], 0.0)
        nc.vector.memset(inp_hr[:, 0:SLAB_OFF], 0.0)
        nc.vector.memset(inp_hr[:, SLAB_OFF + SLAB_ROWS * WP:], 0.0)
        # split slab loads so early conv tiles start before the full load lands
        SPL = 28 * WP
        nc.sync.dma_start(
            out=inp_lr[:, SLAB_OFF:SLAB_OFF + SPL], in_=lr_slab_t[:, 0:SPL])
        nc.sync.dma_start(
            out=inp_hr[:, SLAB_OFF:SLAB_OFF + SPL], in_=hr_slab_t[:, 0:SPL])
        nc.sync.dma_start(
            out=inp_lr[:, SLAB_OFF + SPL:SLAB_OFF + SLAB_ROWS * WP],
            in_=lr_slab_t[:, SPL:])
        nc.sync.dma_start(
            out=inp_hr[:, SLAB_OFF + SPL:SLAB_OFF + SLAB_ROWS * WP],
            in_=hr_slab_t[:, SPL:])

        def _early_out():
            with tc.tile_pool(name="early_out", bufs=1) as pool:
                t = pool.tile([C, NQ], F32)
                nc.vector.memset(t[:], 0.0)
                nc.sync.dma_start(out=out_t[:], in_=t[:])

        # ---------------- offset conv (emitted in per-supertile groups) ------
        ps_off_pool = ctx.enter_context(
            tc.tile_pool(name="ps_off", bufs=1, space="PSUM"))
        ps_offT_pool = ctx.enter_context(
            tc.tile_pool(name="ps_offT", bufs=2, space="PSUM"))

        def emit_conv_tile(ot):
            u0 = SLAB_OFF + (1 + 5 * ot) * WP
            ps = ps_off_pool.tile([18, CONV_N], F32, tag="conv_ps")
            first = True
            for tap in range(NK):
                ky, kx = tap // 3, tap % 3
                shift = (ky - 1) * WP + (kx - 1)
                for c0, img in ((0, inp_lr), (1, inp_hr)):
                    nc.tensor.matmul(
                        ps[:],
                        lhsT_off[:, (c0 * NK + tap) * 18:(c0 * NK + tap + 1) * 18],
                        img[:, u0 + shift:u0 + shift + CONV_N],
                        start=first,
                        stop=(tap == NK - 1 and c0 == 1),
                    )
                    first = False
            nrow = 5 if ot < CONV_NT - 1 else 3
            src = ps[:].rearrange("p (r w) -> p r w", w=WP)[:, 0:nrow, 1:97]
            dst = off_sb[:, ot * 480: ot * 480 + nrow * 96] \
                .rearrange("p (r w) -> p r w", w=96)
            nc.vector.tensor_scalar(dst, src, b_off_sb[:], None, ALU.add)

        def emit_offT(j):
            ps = ps_offT_pool.tile([128, 128], F32, tag="offT_ps")
            nc.tensor.transpose(ps[:, 0:18],
                                off_sb[:, j * 128:(j + 1) * 128],
                                id32[0:18, 0:18])
            nc.scalar.copy(out=offT_sb[:, j * 18:(j + 1) * 18],
                           in_=ps[:, 0:18])

        # ---------------- coordinate math (batched over all tiles) ----------------
        coord = ctx.enter_context(tc.tile_pool(name="coord", bufs=1))
        s_ = coord.tile([C, NT * 18], F32, tag="s_")
        f_ = coord.tile([C, NT * 18], F32, tag="f_")
        d_ = coord.tile([C, NT * 18], F32, tag="d_")
        e_ = coord.tile([C, NT * 18], F32, tag="e_")
        idxf = coord.tile([C, NT * NK], F32, tag="idxf")
        idx16 = coord.tile([C, NT * NK], I16, tag="idx16")

        def ysel(t, sc):   # [C, STT, NK] view of y entries for supertile sc
            return t[:].rearrange("p (t k two) -> p t k two", k=NK, two=2) \
                [:, sc * STT:(sc + 1) * STT, :, 0]

        def xsel(t, sc):
            return t[:].rearrange("p (t k two) -> p t k two", k=NK, two=2) \
                [:, sc * STT:(sc + 1) * STT, :, 1]

        def ysel2(t, sc):  # same, broadcast to the rep-2 weight layout
            return t[:].rearrange("p (t k two) -> p t k two", k=NK, two=2) \
                [:, sc * STT:(sc + 1) * STT, :, 0:1] \
                .broadcast_to([C, STT, NK, 2])

        def xsel2(t, sc):
            return t[:].rearrange("p (t k two) -> p t k two", k=NK, two=2) \
                [:, sc * STT:(sc + 1) * STT, :, 1:2] \
                .broadcast_to([C, STT, NK, 2])

        def w3(t, sc):
            return t[:].rearrange("p (t k) -> p t k", k=NK) \
                [:, sc * STT:(sc + 1) * STT, :]

        def w32(t, sc):
            return t[:].rearrange("p (t k r) -> p t k r", k=NK, r=2) \
                [:, sc * STT:(sc + 1) * STT, :, :]

        # per-supertile chunks so st=0 gathers start while conv continues;
        # conv tiles are emitted just-in-time per chunk (priority order)
        SW = STQ // 16
        CONV_NEED = {0: 3, 1: 5, 2: 8, 3: 10}
        conv_done = 0
        nc.vector.memset(idx16[:], 0)
        for sc in (range(NST) if stage >= 4 else []):
            if stage >= 3:
                for ot in range(conv_done, CONV_NEED[sc]):
                    emit_conv_tile(ot)
                conv_done = CONV_NEED[sc]
            for j in range(sc * STT, (sc + 1) * STT):
                emit_offT(j)
            c2 = slice(sc * STT * 18, (sc + 1) * STT * 18)
            ck = slice(sc * STT * NK, (sc + 1) * STT * NK)
            nc.vector.tensor_add(s_[:, c2], offT_sb[:, c2], base_sb[:, c2])
            nc.vector.tensor_scalar(s_[:, c2], s_[:, c2], CLAMP_LO, CLAMP_HI,
                                    ALU.max, ALU.min)
            nc.vector.tensor_scalar(f_[:, c2], s_[:, c2], -0.5, MAGIC,
                                    ALU.add, ALU.add)
            nc.vector.tensor_scalar(f_[:, c2], f_[:, c2], MAGIC, None,
                                    ALU.subtract)
            nc.vector.tensor_sub(d_[:, c2], s_[:, c2], f_[:, c2])
            nc.vector.tensor_scalar(e_[:, c2], d_[:, c2], -1.0, 1.0,
                                    ALU.mult, ALU.add)
            nc.vector.tensor_mul(w32(w00, sc), ysel2(e_, sc), xsel2(e_, sc))
            nc.vector.tensor_mul(w32(w01, sc), ysel2(e_, sc), xsel2(d_, sc))
            nc.vector.tensor_mul(w32(w10, sc), ysel2(d_, sc), xsel2(e_, sc))
            nc.vector.tensor_mul(w32(w11, sc), ysel2(d_, sc), xsel2(d_, sc))
            nc.vector.tensor_scalar(w3(idxf, sc), ysel(f_, sc), float(WP),
                                    float(WP + 1), ALU.mult, ALU.add)
            nc.vector.tensor_add(w3(idxf, sc), w3(idxf, sc), xsel(f_, sc))
            nc.vector.tensor_copy(out=idx16[:, ck], in_=idxf[:, ck])

            # ---- wrap for dma_gather, SBUF-only (no DRAM round trip) ----
            # idxw[b, (st, k, s)] = idx of pixel q = st*STQ + s*16 + b at
            # tap k.  q = t*128 + p with p = 16g + b and s = t9*8 + g, so
            # group g of idx16's partitions shifts to partitions 0:16
            # (DMA), then a strided DVE copy places it.
            if stage >= 5:
                stw = idxw[0:16, sc * NK * SW:(sc + 1) * NK * SW]
                idxw5 = stw.rearrange("p (k t9 g) -> p k t9 g", k=NK, t9=STT)
                for g in range(8):
                    blk = idxm[0:16, (sc * 8 + g) * STT * NK:
                               (sc * 8 + g + 1) * STT * NK]
                    nc.sync.dma_start(out=blk, in_=idx16[16 * g:16 * (g + 1), ck])
                    srcg = blk.rearrange("p (t9 k) -> p k t9", t9=STT)
                    nc.vector.tensor_copy(out=idxw5[:, :, :, g], in_=srcg)
                # replicate to all 8 16-partition groups (log2 doubling)
                cw = slice(sc * NK * SW, (sc + 1) * NK * SW)
                nc.sync.dma_start(out=idxw[16:32, cw], in_=idxw[0:16, cw])
                nc.sync.dma_start(out=idxw[32:64, cw], in_=idxw[0:32, cw])
                nc.sync.dma_start(out=idxw[64:128, cw], in_=idxw[0:64, cw])

        # ---------------- main loop: gather / combine / transpose / einsum ----------
        gat_pool = ctx.enter_context(tc.tile_pool(name="gat", bufs=3))
        smp_pool = ctx.enter_context(tc.tile_pool(name="smp", bufs=2))
        ps_mm = ctx.enter_context(tc.tile_pool(name="ps_mm", bufs=1, space="PSUM"))
        ps_tr = ctx.enter_context(tc.tile_pool(name="ps_tr", bufs=2, space="PSUM"))
        out_pool = ctx.enter_context(tc.tile_pool(name="outp", bufs=2))

        # each gathered element: imgT2 rows [idx, idx+1] = 4C values
        #   [v00 (y0,x0) | v10 (y0+1,x0) | v01 (y0,x0+1) | v11 (y0+1,x0+1)]
        img_ap = bass.AP(tensor=imgT2_t, offset=0,
                         ap=[[2 * C, NPAD - 1], [1, 4 * C]])

        if stage < 9:
            _early_out()
        for st in (range(NST) if stage >= 6 else []):
            pse = ps_mm.tile([128, STQ], F32, tag="einsum_ps")
            for k in range(NK):
                idx_ap = idxw[:, (st * NK + k) * (STQ // 16):
                              (st * NK + k + 1) * (STQ // 16)]
                g4 = gat_pool.tile([128, STT, 4 * C], FP16, tag="g4")
                nc.gpsimd.dma_gather(g4[:], img_ap, idx_ap, STQ, STQ,
                                     elem_size=4 * C, elem_step=2 * C,
                                     single_packet=False)

                def wview(wt):
                    # [128, STT, C/2, 2]: innermost = the rep-2 pair (step 1)
                    return wt[:].rearrange("p (t k r) -> p t k r", k=NK, r=2) \
                        [:, st * STT:(st + 1) * STT, k:k + 1, :] \
                        .broadcast_to([128, STT, C // 2, 2])

                def r2(ap):
                    return ap.rearrange("p t (h r) -> p t h r", r=2)

                v00 = g4[:, :, 0:C]
                v10 = g4[:, :, C:2 * C]
                v01 = g4[:, :, 2 * C:3 * C]
                v11 = g4[:, :, 3 * C:4 * C]
                t0 = smp_pool.tile([128, STT, C], FP16, tag="t0")
                t1 = smp_pool.tile([128, STT, C], FP16, tag="t1")
                samp = smp_pool.tile([128, STT * C], FP16, tag="samp")
                samp3 = samp[:].rearrange("p (t c) -> p t c", c=C)
                if stage >= 7:
                    nc.vector.tensor_mul(r2(t0[:]), r2(v00), wview(w00))
                    nc.vector.tensor_mul(r2(t1[:]), r2(v01), wview(w01))
                    nc.vector.tensor_add(t0[:], t0[:], t1[:])
                    nc.vector.tensor_mul(r2(t1[:]), r2(v10), wview(w10))
                    nc.vector.tensor_add(t0[:], t0[:], t1[:])
                    nc.vector.tensor_mul(r2(t1[:]), r2(v11), wview(w11))
                    nc.vector.tensor_add(samp3, t0[:], t1[:])
                else:
                    nc.vector.tensor_copy(samp3, v00)

                # transposes batched 4+4+1 into PSUM, copied out in 3 ACT ops
                sampT = smp_pool.tile([128, STT * C], FP16, tag="sampT")
                for grp, glen in ((0, 4), (4, 4), (8, 1)) if stage >= 8 else []:
                    pst = ps_tr.tile([128, 512], FP16, tag="tr_ps")
                    for j2 in range(grp, grp + glen):
                        nc.tensor.transpose(pst[:, (j2 - grp) * C:
                                                 (j2 - grp + 1) * C],
                                            samp[:, j2 * C:(j2 + 1) * C],
                                            id16[:])
                    nc.scalar.copy(out=sampT[:, grp * C:(grp + glen) * C],
                                   in_=pst[:, 0:glen * C])
                for n0 in (range(0, STQ, 512) if stage >= 9 else []):
                    nn = min(512, STQ - n0)
                    nc.tensor.matmul(
                        pse[:, n0:n0 + nn],
                        lhsT_def[:, k * C:(k + 1) * C],
                        sampT[:, n0:n0 + nn],
                        start=(k == 0),
                        stop=(k == NK - 1),
                        skip_group_check=True,
                    )
            if stage >= 9:
                ost = out_pool.tile([128, STQ], F32, tag="ost")
                nc.vector.tensor_scalar(ost[:], pse[:], b_def_sb[:], None, ALU.add)
                nc.sync.dma_start(out=out_t[:, st * STQ:(st + 1) * STQ], in_=ost[:])

    nc.compile()
    if multi_queue:
        _rebalance_swdge_queues(nc)
    return nc


def _rebalance_swdge_queues(nc):
    """Align each SWDGE DMA's queue with its Tile-assigned DMASW sem lane
    (queue = lane % 4) so a given SWDGE global sem is only ever updated from
    one queue, while gathers spread across all 4 queue contexts."""
    import re
    for blk in nc.m.functions[0].blocks:
        for inst in blk.instructions:
            nm = type(inst).__name__
            is_gather = 'DMAGather' in nm
            is_pool_copy = (nm == 'InstDMACopy'
                            and str(getattr(inst, 'queue', '')).startswith('qPoolDynamic'))
            if not (is_gather or is_pool_copy):
                continue
            si = inst.sync_info
            lane = None
            if si:
                for u in si.on_update:
                    m = re.match(r'DMASW(\d+)', str(getattr(u, 'ant_name', '') or ''))
                    if m:
                        lane = int(m.group(1))
                        break
            if lane is None:
                continue
            q = lane % 4
            if is_gather:
                inst.queue_num = q
            else:
                inst.queue = 'qPoolDynamic' + ('' if q == 0 else str(q))


# ---------------- host-side prep ----------------

def _pack_weights(w_off, w_def, b_off, b_def):
    # [i, (c0, tap, o)] and [i, (k, o)] -- match SBUF lhsT layouts exactly
    w_off_p = np.ascontiguousarray(
        w_off.reshape(18, 2, C, NK).transpose(2, 1, 3, 0).reshape(C, 2 * NK * 18)
        .astype(np.float16))
    w_def_p = np.ascontiguousarray(
        w_def.reshape(C, C, NK).transpose(1, 2, 0).reshape(C, NK * C)
        .astype(np.float16))
    b_off_p = np.ascontiguousarray(b_off.reshape(18, 1).astype(np.float32))
    b_def_p = np.ascontiguousarray(b_def.reshape(C, 1).astype(np.float32))
    return w_off_p, w_def_p, b_off_p, b_def_p


def _base_grid(h):
    # base[t, tile, 2k]   = y + (k//3 - 1)   with q = h*NQ + tile*128 + t
    # base[t, tile, 2k+1] = x + (k%3 - 1)
    q = h * NQ + np.arange(NT)[None, :] * 128 + np.arange(128)[:, None]  # [128, NT]
    y = (q // W).astype(np.float32)
    x = (q % W).astype(np.float32)
    k = np.arange(NK)
    ky = (k // 3 - 1).astype(np.float32)
    kx = (k % 3 - 1).astype(np.float32)
    base = np.zeros((128, NT, 18), np.float32)
    base[:, :, 0::2] = y[:, :, None] + ky[None, None, :]
    base[:, :, 1::2] = x[:, :, None] + kx[None, None, :]
    return base


def _slab(img_b, h):
    # rows 48h-1 .. 48h+48, zero padded rows and columns, width WP, fp16
    out = np.zeros((C, SLAB_ROWS, WP), np.float16)
    r0 = HLF * h - 1
    for r in range(SLAB_ROWS):
        g = r0 + r
        if 0 <= g < H:
            out[:, r, 1:97] = img_b[:, g, :]
    return out.reshape(C, SLAB_ROWS * WP)


def _imgT2(img_b):
    # y-pair transposed padded image: row r=(y*WP+x) of the padded image,
    # imgT2[r] = [pad[:, y, x], pad[:, y+1, x]]  -> [NPAD, 2C] fp16
    pad = np.zeros((C, 101, WP), np.float16)
    pad[:, 1:97, 1:97] = img_b
    flat = np.ascontiguousarray(pad.reshape(C, 101 * WP).T)  # [9898, C]
    return np.ascontiguousarray(
        np.concatenate([flat[:NPAD], flat[WP:WP + NPAD]], axis=1))


_NC_CACHE = {}


def get_nc(num_devices=8, enable_asserts=True, stage=99, multi_queue=True):
    key = (num_devices, enable_asserts, stage, multi_queue)
    if key not in _NC_CACHE:
        _NC_CACHE[key] = _build_program(num_devices, enable_asserts,
                                        stage, multi_queue)
    return _NC_CACHE[key]


def make_in_maps(lr_features, hr_features, w_off, b_off, w_def, b_def):
    lr = np.asarray(lr_features, np.float32)
    hr = np.asarray(hr_features, np.float32)
    w_off_p, w_def_p, b_off_p, b_def_p = _pack_weights(
        np.asarray(w_off, np.float32), np.asarray(w_def, np.float32),
        np.asarray(b_off, np.float32), np.asarray(b_def, np.float32))
    id16 = np.eye(128, dtype=np.float16)
    id32 = np.eye(128, dtype=np.float32)
    bases = [_base_grid(0), _base_grid(1)]
    lr16 = lr.astype(np.float16)
    hr16 = hr.astype(np.float16)
    img2 = [_imgT2(hr16[b]) for b in range(B)]
    in_maps = []
    for core in range(8):
        b, h = core // 2, core % 2
        in_maps.append({
            "imgT2": img2[b],
            "lr_slab": _slab(lr16[b], h),
            "hr_slab": _slab(hr16[b], h),
            "w_off_p": w_off_p,
            "w_def_p": w_def_p,
            "b_off_p": b_off_p,
            "b_def_p": b_def_p,
            "base_p": bases[h],
            "ident16": id16,
            "ident32": id32,
        })
    return in_maps


def run_cores(in_maps, trace=False):
    nc = get_nc()
    res = run_bass_kernel_spmd(nc, in_maps, list(range(8)), trace=trace)
    return res


def assemble(results):
    out = np.zeros((B, C, H, W), np.float32)
    for core in range(8):
        b, h = core // 2, core % 2
        out[b, :, HLF * h:HLF * (h + 1), :] = \
            results[core]["out"].reshape(C, HLF, W)
    return out


def kernel(lr_features, hr_features, w_off, b_off, w_def, b_def):
    in_maps = make_in_maps(lr_features, hr_features, w_off, b_off, w_def, b_def)
    res = run_cores(in_maps, trace=False)
    return assemble(res.results)


# revision 34
# speedup vs baseline: 1.3984x; 1.3984x over previous
"""Trainium2 Bass kernel for DeformableConvBlock.

Reference computation:
    inp    = concat([lr, hr], axis=1)              # [B, 256, 96, 96]
    offset = conv3x3(inp, w_off, b_off)            # [B, 18, 96, 96]
    out    = deform_conv3x3(hr, offset, w_def, b_def)

Sharding: 8 cores = 4 batches x 2 half-images (48 output rows each).
Each core:
  - computes its offset conv slice (18 matmuls / tile, shifted-window APs)
  - computes sample coords/weights on DVE (pixel-partition layout,
    magic-number floor; clamp to [-1+eps, 96] + zero border reproduces
    torchvision OOB semantics exactly)
  - bilinear sampling via ONE dma_gather per tap from a host-built
    y-pair transposed padded fp16 image (each 1KB element holds all
    4 bilinear neighbors: rows (y0,x0),(y0+1,x0),(y0,x0+1),(y0+1,x0+1))
  - weighted 4-neighbor combine on DVE via tensor_scalar +
    scalar_tensor_tensor with per-partition (per-pixel) weight scalars
  - PE-transposes combined samples to channel-partition, einsum as 9
    PSUM-accumulated matmuls per 1152-pixel supertile
"""

import os
import numpy as np
from contextlib import ExitStack

import concourse.bass as bass
import concourse.tile as tile
from concourse import bacc, mybir
from concourse.bass_utils import run_bass_kernel_spmd

F32 = mybir.dt.float32
FP16 = mybir.dt.float16
I16 = mybir.dt.int16
ALU = mybir.AluOpType

# ---- geometry (hardcoded for the 4x128x96x96 problem) ----
B, C, H, W = 4, 128, 96, 96
WP = H + 2                    # 98  padded width
NPIMG = WP * WP               # 9604 padded pixels
NPAD = 76 * 128               # 9728 imgT2 rows (covers 9604 + slack)
HLF = H // 2                  # 48 output rows per core
NQ = HLF * W                  # 4608 output pixels per core
NT = NQ // 128                # 36 pixel tiles
NST = 4                       # supertiles
STT = NT // NST               # 9 tiles per supertile
STQ = STT * 128               # 1152 pixels per supertile
SLAB_ROWS = 50                # conv input rows per core (48 + halo)
SLAB_OFF = WP                 # leading zero row in conv input buffers
SLAB_ALLOC = SLAB_OFF + SLAB_ROWS * WP + 220   # slack for window overread
MAGIC = 12582912.0            # 1.5 * 2**23
CLAMP_LO = -0.9999999
CLAMP_HI = 96.0
NK = 9                        # taps
CONV_NT = 10                  # offset-conv output tiles (5 rows each)
CONV_N = 5 * WP               # 490


def _build_program(num_devices=8, enable_asserts=True, stage=99,
                   multi_queue=True):
    nc = bacc.Bacc(
        "TRN2",
        target_bir_lowering=False,
        debug=False,
        enable_asserts=enable_asserts,
        num_devices=num_devices,
        num_swdge_queues=4 if multi_queue else 1,
    )

    def inp(name, shape, dt):
        return nc.dram_tensor(name, shape, dt, kind="ExternalInput")

    imgT2_t = inp("imgT2", [NPAD, 2 * C], FP16)
    lr_slab_t = inp("lr_slab", [C, SLAB_ROWS * WP], FP16)
    hr_slab_t = inp("hr_slab", [C, SLAB_ROWS * WP], FP16)
    w_off_t = inp("w_off_p", [C, 2 * NK * 18], FP16)
    w_def_t = inp("w_def_p", [C, NK * C], FP16)
    b_off_t = inp("b_off_p", [18, 1], F32)
    b_def_t = inp("b_def_p", [C, 1], F32)
    base_t = inp("base_p", [C, NT, 18], F32)
    id16_t = inp("ident16", [128, 128], FP16)
    id32_t = inp("ident32", [128, 128], F32)

    out_t = nc.dram_tensor("out", [C, NQ], F32, kind="ExternalOutput")

    with tile.TileContext(nc) as tc, ExitStack() as ctx:
        persist = ctx.enter_context(tc.tile_pool(name="persist", bufs=1))

        # ---------------- persistent SBUF tensors ----------------
        inp_lr = persist.tile([C, SLAB_ALLOC], FP16, tag="inp_lr")
        inp_hr = persist.tile([C, SLAB_ALLOC], FP16, tag="inp_hr")
        lhsT_off = persist.tile([C, 2 * NK * 18], FP16, tag="lhsT_off")
        lhsT_def = persist.tile([C, NK * C], FP16, tag="lhsT_def")
        b_off_sb = persist.tile([18, 1], F32, tag="b_off_sb")
        b_def_sb = persist.tile([C, 1], F32, tag="b_def_sb")
        base_sb = persist.tile([C, NT * 18], F32, tag="base_sb")
        id16 = persist.tile([128, 128], FP16, tag="id16")
        id32 = persist.tile([128, 128], F32, tag="id32")
        off_sb = persist.tile([18, NQ], F32, tag="off_sb")
        offT_sb = persist.tile([C, NT * 18], F32, tag="offT_sb")
        # rep-2 weight planes: each weight stored twice contiguously so the
        # broadcast view's innermost dim is step-1 packed pairs (DVE 2x mode)
        w00 = persist.tile([C, NT * NK * 2], FP16, tag="w00")
        w01 = persist.tile([C, NT * NK * 2], FP16, tag="w01")
        w10 = persist.tile([C, NT * NK * 2], FP16, tag="w10")
        w11 = persist.tile([C, NT * NK * 2], FP16, tag="w11")
        idxw = persist.tile([128, NK * NST * (STQ // 16)], I16, tag="idxw")
        idxm = persist.tile([16, 8 * NT * NK], I16, tag="idxm")

        # ---------------- load constants / weights / slabs ----------------
        nc.sync.dma_start(out=lhsT_off[:], in_=w_off_t[:])
        nc.sync.dma_start(out=lhsT_def[:], in_=w_def_t[:])
        nc.sync.dma_start(out=b_off_sb[:], in_=b_off_t[:])
        nc.sync.dma_start(out=b_def_sb[:], in_=b_def_t[:])
        nc.sync.dma_start(out=base_sb[:], in_=base_t[:].rearrange("p t c -> p (t c)"))
        nc.sync.dma_start(out=id16[:], in_=id16_t[:])
        nc.sync.dma_start(out=id32[:], in_=id32_t[:])

        nc.vector.memset(inp_lr[:, 0:SLAB_OFF], 0.0)
        nc.vector.memset(inp_lr[:, SLAB_OFF + SLAB_ROWS * WP:], 0.0)
        nc.vector.memset(inp_hr[:, 0:SLAB_OFF], 0.0)
        nc.vector.memset(inp_hr[:, SLAB_OFF + SLAB_ROWS * WP:], 0.0)
        # split slab loads so early conv tiles start before the full load lands
        SPL = 28 * WP
        nc.sync.dma_start(
            out=inp_lr[:, SLAB_OFF:SLAB_OFF + SPL], in_=lr_slab_t[:, 0:SPL])
        nc.sync.dma_start(
            out=inp_hr[:, SLAB_OFF:SLAB_OFF + SPL], in_=hr_slab_t[:, 0:SPL])
        nc.sync.dma_start(
            out=inp_lr[:, SLAB_OFF + SPL:SLAB_OFF + SLAB_ROWS * WP],
            in_=lr_slab_t[:, SPL:])
        nc.sync.dma_start(
            out=inp_hr[:, SLAB_OFF + SPL:SLAB_OFF + SLAB_ROWS * WP],
            in_=hr_slab_t[:, SPL:])

        def _early_out():
            with tc.tile_pool(name="early_out", bufs=1) as pool:
                t = pool.tile([C, NQ], F32)
                nc.vector.memset(t[:], 0.0)
                nc.sync.dma_start(out=out_t[:], in_=t[:])

        # ---------------- offset conv (emitted in per-supertile groups) ------
        ps_off_pool = ctx.enter_context(
            tc.tile_pool(name="ps_off", bufs=1, space="PSUM"))
        ps_offT_pool = ctx.enter_context(
            tc.tile_pool(name="ps_offT", bufs=2, space="PSUM"))

        def emit_conv_tile(ot):
            u0 = SLAB_OFF + (1 + 5 * ot) * WP
            ps = ps_off_pool.tile([18, CONV_N], F32, tag="conv_ps")
            first = True
            for tap in range(NK):
                ky, kx = tap // 3, tap % 3
                shift = (ky - 1) * WP + (kx - 1)
                for c0, img in ((0, inp_lr), (1, inp_hr)):
                    nc.tensor.matmul(
                        ps[:],
                        lhsT_off[:, (c0 * NK + tap) * 18:(c0 * NK + tap + 1) * 18],
                        img[:, u0 + shift:u0 + shift + CONV_N],
                        start=first,
                        stop=(tap == NK - 1 and c0 == 1),
                    )
                    first = False
            nrow = 5 if ot < CONV_NT - 1 else 3
            src = ps[:].rearrange("p (r w) -> p r w", w=WP)[:, 0:nrow, 1:97]
            dst = off_sb[:, ot * 480: ot * 480 + nrow * 96] \
                .rearrange("p (r w) -> p r w", w=96)
            nc.vector.tensor_scalar(dst, src, b_off_sb[:], None, ALU.add)

        def emit_offT(j):
            ps = ps_offT_pool.tile([128, 128], F32, tag="offT_ps")
            nc.tensor.transpose(ps[:, 0:18],
                                off_sb[:, j * 128:(j + 1) * 128],
                                id32[0:18, 0:18])
            nc.scalar.copy(out=offT_sb[:, j * 18:(j + 1) * 18],
                           in_=ps[:, 0:18])

        # ---------------- coordinate math (batched over all tiles) ----------------
        coord = ctx.enter_context(tc.tile_pool(name="coord", bufs=1))
        s_ = coord.tile([C, NT * 18], F32, tag="s_")
        f_ = coord.tile([C, NT * 18], F32, tag="f_")
        d_ = coord.tile([C, NT * 18], F32, tag="d_")
        e_ = coord.tile([C, NT * 18], F32, tag="e_")
        idxf = coord.tile([C, NT * NK], F32, tag="idxf")
        idx16 = coord.tile([C, NT * NK], I16, tag="idx16")

        def ysel(t, sc):   # [C, STT, NK] view of y entries for supertile sc
            return t[:].rearrange("p (t k two) -> p t k two", k=NK, two=2) \
                [:, sc * STT:(sc + 1) * STT, :, 0]

        def xsel(t, sc):
            return t[:].rearrange("p (t k two) -> p t k two", k=NK, two=2) \
                [:, sc * STT:(sc + 1) * STT, :, 1]

        def ysel2(t, sc):  # same, broadcast to the rep-2 weight layout
            return t[:].rearrange("p (t k two) -> p t k two", k=NK, two=2) \
                [:, sc * STT:(sc + 1) * STT, :, 0:1] \
                .broadcast_to([C, STT, NK, 2])

        def xsel2(t, sc):
            return t[:].rearrange("p (t k two) -> p t k two", k=NK, two=2) \
                [:, sc * STT:(sc + 1) * STT, :, 1:2] \
                .broadcast_to([C, STT, NK, 2])

        def w3(t, sc):
            return t[:].rearrange("p (t k) -> p t k", k=NK) \
                [:, sc * STT:(sc + 1) * STT, :]

        def w32(t, sc):
            return t[:].rearrange("p (t k r) -> p t k r", k=NK, r=2) \
                [:, sc * STT:(sc + 1) * STT, :, :]

        # per-supertile chunks so st=0 gathers start while conv continues;
        # conv tiles are emitted just-in-time per chunk (priority order)
        SW = STQ // 16
        CONV_NEED = {0: 3, 1: 5, 2: 8, 3: 10}
        conv_done = 0
        nc.vector.memset(idx16[:], 0)
        for sc in (range(NST) if stage >= 4 else []):
            if stage >= 3:
                for ot in range(conv_done, CONV_NEED[sc]):
                    emit_conv_tile(ot)
                conv_done = CONV_NEED[sc]
            for j in range(sc * STT, (sc + 1) * STT):
                emit_offT(j)
            c2 = slice(sc * STT * 18, (sc + 1) * STT * 18)
            ck = slice(sc * STT * NK, (sc + 1) * STT * NK)
            nc.vector.tensor_add(s_[:, c2], offT_sb[:, c2], base_sb[:, c2])
            nc.vector.tensor_scalar(s_[:, c2], s_[:, c2], CLAMP_LO, CLAMP_HI,
                                    ALU.max, ALU.min)
            nc.vector.tensor_scalar(f_[:, c2], s_[:, c2], -0.5, MAGIC,
                                    ALU.add, ALU.add)
            nc.vector.tensor_scalar(f_[:, c2], f_[:, c2], MAGIC, None,
                                    ALU.subtract)
            nc.vector.tensor_sub(d_[:, c2], s_[:, c2], f_[:, c2])
            nc.vector.tensor_scalar(e_[:, c2], d_[:, c2], -1.0, 1.0,
                                    ALU.mult, ALU.add)
            nc.vector.tensor_mul(w32(w00, sc), ysel2(e_, sc), xsel2(e_, sc))
            nc.vector.tensor_mul(w32(w01, sc), ysel2(e_, sc), xsel2(d_, sc))
            nc.vector.tensor_mul(w32(w10, sc), ysel2(d_, sc), xsel2(e_, sc))
            nc.vector.tensor_mul(w32(w11, sc), ysel2(d_, sc), xsel2(d_, sc))
            nc.vector.tensor_scalar(w3(idxf, sc), ysel(f_, sc), float(WP),
                                    float(WP + 1), ALU.mult, ALU.add)
            nc.vector.tensor_add(w3(idxf, sc), w3(idxf, sc), xsel(f_, sc))
            nc.vector.tensor_copy(out=idx16[:, ck], in_=idxf[:, ck])

            # ---- wrap for dma_gather, SBUF-only (no DRAM round trip) ----
            # idxw[b, (st, k, s)] = idx of pixel q = st*STQ + s*16 + b at
            # tap k.  q = t*128 + p with p = 16g + b and s = t9*8 + g, so
            # group g of idx16's partitions shifts to partitions 0:16
            # (DMA), then a strided DVE copy places it.
            if stage >= 5:
                stw = idxw[0:16, sc * NK * SW:(sc + 1) * NK * SW]
                idxw5 = stw.rearrange("p (k t9 g) -> p k t9 g", k=NK, t9=STT)
                for g in range(8):
                    blk = idxm[0:16, (sc * 8 + g) * STT * NK:
                               (sc * 8 + g + 1) * STT * NK]
                    nc.sync.dma_start(out=blk, in_=idx16[16 * g:16 * (g + 1), ck])
                    srcg = blk.rearrange("p (t9 k) -> p k t9", t9=STT)
                    nc.vector.tensor_copy(out=idxw5[:, :, :, g], in_=srcg)
                # replicate to all 8 16-partition groups (log2 doubling)
                cw = slice(sc * NK * SW, (sc + 1) * NK * SW)
                nc.sync.dma_start(out=idxw[16:32, cw], in_=idxw[0:16, cw])
                nc.sync.dma_start(out=idxw[32:64, cw], in_=idxw[0:32, cw])
                nc.sync.dma_start(out=idxw[64:128, cw], in_=idxw[0:64, cw])

        # ---------------- main loop: gather / combine / transpose / einsum ----------
        gat_pool = ctx.enter_context(tc.tile_pool(name="gat", bufs=6))
        smp_pool = ctx.enter_context(tc.tile_pool(name="smp", bufs=2))
        ps_mm = ctx.enter_context(tc.tile_pool(name="ps_mm", bufs=1, space="PSUM"))
        ps_tr = ctx.enter_context(tc.tile_pool(name="ps_tr", bufs=2, space="PSUM"))
        out_pool = ctx.enter_context(tc.tile_pool(name="outp", bufs=2))

        # each gathered element: imgT2 rows [idx, idx+1] = 4C values
        #   [v00 (y0,x0) | v10 (y0+1,x0) | v01 (y0,x0+1) | v11 (y0+1,x0+1)]
        img_ap = bass.AP(tensor=imgT2_t, offset=0,
                         ap=[[2 * C, NPAD - 1], [1, 4 * C]])

        if stage < 9:
            _early_out()
        for st in (range(NST) if stage >= 6 else []):
            pse = ps_mm.tile([128, STQ], F32, tag="einsum_ps")
            for k in range(NK):
                idx_ap = idxw[:, (st * NK + k) * (STQ // 16):
                              (st * NK + k + 1) * (STQ // 16)]
                g4 = gat_pool.tile([128, STT, 4 * C], FP16, tag="g4")
                nc.gpsimd.dma_gather(g4[:], img_ap, idx_ap, STQ, STQ,
                                     elem_size=4 * C, elem_step=2 * C,
                                     single_packet=False)

                def wview(wt):
                    # [128, STT, C/2, 2]: innermost = the rep-2 pair (step 1)
                    return wt[:].rearrange("p (t k r) -> p t k r", k=NK, r=2) \
                        [:, st * STT:(st + 1) * STT, k:k + 1, :] \
                        .broadcast_to([128, STT, C // 2, 2])

                def r2(ap):
                    return ap.rearrange("p t (h r) -> p t h r", r=2)

                v00 = g4[:, :, 0:C]
                v10 = g4[:, :, C:2 * C]
                v01 = g4[:, :, 2 * C:3 * C]
                v11 = g4[:, :, 3 * C:4 * C]
                t0 = smp_pool.tile([128, STT, C], FP16, tag="t0")
                t1 = smp_pool.tile([128, STT, C], FP16, tag="t1")
                samp = smp_pool.tile([128, STT * C], FP16, tag="samp")
                samp3 = samp[:].rearrange("p (t c) -> p t c", c=C)
                if stage >= 7:
                    nc.vector.tensor_mul(r2(t0[:]), r2(v00), wview(w00))
                    nc.vector.tensor_mul(r2(t1[:]), r2(v01), wview(w01))
                    nc.vector.tensor_add(t0[:], t0[:], t1[:])
                    nc.vector.tensor_mul(r2(t1[:]), r2(v10), wview(w10))
                    nc.vector.tensor_add(t0[:], t0[:], t1[:])
                    nc.vector.tensor_mul(r2(t1[:]), r2(v11), wview(w11))
                    nc.vector.tensor_add(samp3, t0[:], t1[:])
                else:
                    nc.vector.tensor_copy(samp3, v00)

                # transposes batched 4+4+1 into PSUM, copied out in 3 ACT ops
                sampT = smp_pool.tile([128, STT * C], FP16, tag="sampT")
                for grp, glen in ((0, 4), (4, 4), (8, 1)) if stage >= 8 else []:
                    pst = ps_tr.tile([128, 512], FP16, tag="tr_ps")
                    for j2 in range(grp, grp + glen):
                        nc.tensor.transpose(pst[:, (j2 - grp) * C:
                                                 (j2 - grp + 1) * C],
                                            samp[:, j2 * C:(j2 + 1) * C],
                                            id16[:])
                    nc.scalar.copy(out=sampT[:, grp * C:(grp + glen) * C],
                                   in_=pst[:, 0:glen * C])
                for n0 in (range(0, STQ, 512) if stage >= 9 else []):
                    nn = min(512, STQ - n0)
                    nc.tensor.matmul(
                        pse[:, n0:n0 + nn],
                        lhsT_def[:, k * C:(k + 1) * C],
                        sampT[:, n0:n0 + nn],
                        start=(k == 0),
                        stop=(k == NK - 1),
                        skip_group_check=True,
                    )
            if stage >= 9:
                ost = out_pool.tile([128, STQ], F32, tag="ost")
                nc.vector.tensor_scalar(ost[:], pse[:], b_def_sb[:], None, ALU.add)
                nc.sync.dma_start(out=out_t[:, st * STQ:(st + 1) * STQ], in_=ost[:])

    nc.compile()
    if multi_queue:
        _rebalance_swdge_queues(nc)
    return nc


def _rebalance_swdge_queues(nc):
    """Align each SWDGE DMA's queue with its Tile-assigned DMASW sem lane
    (queue = lane % 4) so a given SWDGE global sem is only ever updated from
    one queue, while gathers spread across all 4 queue contexts."""
    import re
    for blk in nc.m.functions[0].blocks:
        for inst in blk.instructions:
            nm = type(inst).__name__
            is_gather = 'DMAGather' in nm
            is_pool_copy = (nm == 'InstDMACopy'
                            and str(getattr(inst, 'queue', '')).startswith('qPoolDynamic'))
            if not (is_gather or is_pool_copy):
                continue
            si = inst.sync_info
            lane = None
            if si:
                for u in si.on_update:
                    m = re.match(r'DMASW(\d+)', str(getattr(u, 'ant_name', '') or ''))
                    if m:
                        lane = int(m.group(1))
                        break
            if lane is None:
                continue
            q = lane % 4
            if is_gather:
                inst.queue_num = q
            else:
                inst.queue = 'qPoolDynamic' + ('' if q == 0 else str(q))


# ---------------- host-side prep ----------------

def _pack_weights(w_off, w_def, b_off, b_def):
    # [i, (c0, tap, o)] and [i, (k, o)] -- match SBUF lhsT layouts exactly
    w_off_p = np.ascontiguousarray(
        w_off.reshape(18, 2, C, NK).transpose(2, 1, 3, 0).reshape(C, 2 * NK * 18)
        .astype(np.float16))
    w_def_p = np.ascontiguousarray(
        w_def.reshape(C, C, NK).transpose(1, 2, 0).reshape(C, NK * C)
        .astype(np.float16))
    b_off_p = np.ascontiguousarray(b_off.reshape(18, 1).astype(np.float32))
    b_def_p = np.ascontiguousarray(b_def.reshape(C, 1).astype(np.float32))
    return w_off_p, w_def_p, b_off_p, b_def_p


def _base_grid(h):
    # base[t, tile, 2k]   = y + (k//3 - 1)   with q = h*NQ + tile*128 + t
    # base[t, tile, 2k+1] = x + (k%3 - 1)
    q = h * NQ + np.arange(NT)[None, :] * 128 + np.arange(128)[:, None]  # [128, NT]
    y = (q // W).astype(np.float32)
    x = (q % W).astype(np.float32)
    k = np.arange(NK)
    ky = (k // 3 - 1).astype(np.float32)
    kx = (k % 3 - 1).astype(np.float32)
    base = np.zeros((128, NT, 18), np.float32)
    base[:, :, 0::2] = y[:, :, None] + ky[None, None, :]
    base[:, :, 1::2] = x[:, :, None] + kx[None, None, :]
    return base


def _slab(img_b, h):
    # rows 48h-1 .. 48h+48, zero padded rows and columns, width WP, fp16
    out = np.zeros((C, SLAB_ROWS, WP), np.float16)
    r0 = HLF * h - 1
    for r in range(SLAB_ROWS):
        g = r0 + r
        if 0 <= g < H:
            out[:, r, 1:97] = img_b[:, g, :]
    return out.reshape(C, SLAB_ROWS * WP)


def _imgT2(img_b):
    # y-pair transposed padded image: row r=(y*WP+x) of the padded image,
    # imgT2[r] = [pad[:, y, x], pad[:, y+1, x]]  -> [NPAD, 2C] fp16
    pad = np.zeros((C, 101, WP), np.float16)
    pad[:, 1:97, 1:97] = img_b
    flat = np.ascontiguousarray(pad.reshape(C, 101 * WP).T)  # [9898, C]
    return np.ascontiguousarray(
        np.concatenate([flat[:NPAD], flat[WP:WP + NPAD]], axis=1))


_NC_CACHE = {}


def get_nc(num_devices=8, enable_asserts=True, stage=99, multi_queue=True):
    key = (num_devices, enable_asserts, stage, multi_queue)
    if key not in _NC_CACHE:
        _NC_CACHE[key] = _build_program(num_devices, enable_asserts,
                                        stage, multi_queue)
    return _NC_CACHE[key]


def make_in_maps(lr_features, hr_features, w_off, b_off, w_def, b_def):
    lr = np.asarray(lr_features, np.float32)
    hr = np.asarray(hr_features, np.float32)
    w_off_p, w_def_p, b_off_p, b_def_p = _pack_weights(
        np.asarray(w_off, np.float32), np.asarray(w_def, np.float32),
        np.asarray(b_off, np.float32), np.asarray(b_def, np.float32))
    id16 = np.eye(128, dtype=np.float16)
    id32 = np.eye(128, dtype=np.float32)
    bases = [_base_grid(0), _base_grid(1)]
    lr16 = lr.astype(np.float16)
    hr16 = hr.astype(np.float16)
    img2 = [_imgT2(hr16[b]) for b in range(B)]
    in_maps = []
    for core in range(8):
        b, h = core // 2, core % 2
        in_maps.append({
            "imgT2": img2[b],
            "lr_slab": _slab(lr16[b], h),
            "hr_slab": _slab(hr16[b], h),
            "w_off_p": w_off_p,
            "w_def_p": w_def_p,
            "b_off_p": b_off_p,
            "b_def_p": b_def_p,
            "base_p": bases[h],
            "ident16": id16,
            "ident32": id32,
        })
    return in_maps


def run_cores(in_maps, trace=False):
    nc = get_nc()
    res = run_bass_kernel_spmd(nc, in_maps, list(range(8)), trace=trace)
    return res


def assemble(results):
    out = np.zeros((B, C, H, W), np.float32)
    for core in range(8):
        b, h = core // 2, core % 2
        out[b, :, HLF * h:HLF * (h + 1), :] = \
            results[core]["out"].reshape(C, HLF, W)
    return out


def kernel(lr_features, hr_features, w_off, b_off, w_def, b_def):
    in_maps = make_in_maps(lr_features, hr_features, w_off, b_off, w_def, b_def)
    res = run_cores(in_maps, trace=False)
    return assemble(res.results)


# revision 35
# speedup vs baseline: 1.5322x; 1.0957x over previous
"""Trainium2 Bass kernel for DeformableConvBlock.

Reference computation:
    inp    = concat([lr, hr], axis=1)              # [B, 256, 96, 96]
    offset = conv3x3(inp, w_off, b_off)            # [B, 18, 96, 96]
    out    = deform_conv3x3(hr, offset, w_def, b_def)

Sharding: 8 cores = 4 batches x 2 half-images (48 output rows each).
Each core:
  - computes its offset conv slice (18 matmuls / tile, shifted-window APs)
  - computes sample coords/weights on DVE (pixel-partition layout,
    magic-number floor; clamp to [-1+eps, 96] + zero border reproduces
    torchvision OOB semantics exactly)
  - bilinear sampling via ONE dma_gather per tap from a host-built
    y-pair transposed padded fp16 image (each 1KB element holds all
    4 bilinear neighbors: rows (y0,x0),(y0+1,x0),(y0,x0+1),(y0+1,x0+1))
  - weighted 4-neighbor combine on DVE via tensor_scalar +
    scalar_tensor_tensor with per-partition (per-pixel) weight scalars
  - PE-transposes combined samples to channel-partition, einsum as 9
    PSUM-accumulated matmuls per 1152-pixel supertile
"""

import os
import numpy as np
from contextlib import ExitStack

import concourse.bass as bass
import concourse.tile as tile
from concourse import bacc, mybir
from concourse.bass_utils import run_bass_kernel_spmd

F32 = mybir.dt.float32
FP16 = mybir.dt.float16
I16 = mybir.dt.int16
ALU = mybir.AluOpType

# ---- geometry (hardcoded for the 4x128x96x96 problem) ----
B, C, H, W = 4, 128, 96, 96
WP = H + 2                    # 98  padded width
NPIMG = WP * WP               # 9604 padded pixels
NPAD = 76 * 128               # 9728 imgT2 rows (covers 9604 + slack)
HLF = H // 2                  # 48 output rows per core
NQ = HLF * W                  # 4608 output pixels per core
NT = NQ // 128                # 36 pixel tiles
NST = 4                       # supertiles
STT = NT // NST               # 9 tiles per supertile
STQ = STT * 128               # 1152 pixels per supertile
SLAB_ROWS = 50                # conv input rows per core (48 + halo)
SLAB_OFF = WP                 # leading zero row in conv input buffers
SLAB_ALLOC = SLAB_OFF + SLAB_ROWS * WP + 220   # slack for window overread
MAGIC = 12582912.0            # 1.5 * 2**23
CLAMP_LO = -0.9999999
CLAMP_HI = 96.0
NK = 9                        # taps
CONV_NT = 10                  # offset-conv output tiles (5 rows each)
CONV_N = 5 * WP               # 490


def _build_program(num_devices=8, enable_asserts=True, stage=99,
                   multi_queue=True):
    nc = bacc.Bacc(
        "TRN2",
        target_bir_lowering=False,
        debug=False,
        enable_asserts=enable_asserts,
        num_devices=num_devices,
        num_swdge_queues=4 if multi_queue else 1,
    )

    def inp(name, shape, dt):
        return nc.dram_tensor(name, shape, dt, kind="ExternalInput")

    imgT2_t = inp("imgT2", [NPAD, 2 * C], FP16)
    lr_slab_t = inp("lr_slab", [C, SLAB_ROWS * WP], FP16)
    hr_slab_t = inp("hr_slab", [C, SLAB_ROWS * WP], FP16)
    w_off_t = inp("w_off_p", [C, 2 * NK * 18], FP16)
    w_def_t = inp("w_def_p", [C, NK * C], FP16)
    b_off_t = inp("b_off_p", [18, 1], F32)
    b_def_t = inp("b_def_p", [C, 1], F32)
    base_t = inp("base_p", [C, NT, 18], F32)
    id16_t = inp("ident16", [128, 128], FP16)
    id32_t = inp("ident32", [128, 128], F32)

    out_t = nc.dram_tensor("out", [C, NQ], F32, kind="ExternalOutput")

    with tile.TileContext(nc) as tc, ExitStack() as ctx:
        persist = ctx.enter_context(tc.tile_pool(name="persist", bufs=1))

        # ---------------- persistent SBUF tensors ----------------
        inp_lr = persist.tile([C, SLAB_ALLOC], FP16, tag="inp_lr")
        inp_hr = persist.tile([C, SLAB_ALLOC], FP16, tag="inp_hr")
        lhsT_off = persist.tile([C, 2 * NK * 18], FP16, tag="lhsT_off")
        lhsT_def = persist.tile([C, NK * C], FP16, tag="lhsT_def")
        b_off_sb = persist.tile([18, 1], F32, tag="b_off_sb")
        b_def_sb = persist.tile([C, 1], F32, tag="b_def_sb")
        base_sb = persist.tile([C, NT * 18], F32, tag="base_sb")
        id16 = persist.tile([128, 128], FP16, tag="id16")
        id32 = persist.tile([128, 128], F32, tag="id32")
        off_sb = persist.tile([18, NQ], F32, tag="off_sb")
        offT_sb = persist.tile([C, NT * 18], F32, tag="offT_sb")
        # rep-2 weight planes: each weight stored twice contiguously so the
        # broadcast view's innermost dim is step-1 packed pairs (DVE 2x mode)
        w00 = persist.tile([C, NT * NK * 2], FP16, tag="w00")
        w01 = persist.tile([C, NT * NK * 2], FP16, tag="w01")
        w10 = persist.tile([C, NT * NK * 2], FP16, tag="w10")
        w11 = persist.tile([C, NT * NK * 2], FP16, tag="w11")
        idxw = persist.tile([128, NK * NST * (STQ // 16)], I16, tag="idxw")
        idxm = persist.tile([16, 8 * NT * NK], I16, tag="idxm")

        # ---------------- load constants / weights / slabs ----------------
        nc.sync.dma_start(out=lhsT_off[:], in_=w_off_t[:])
        nc.sync.dma_start(out=lhsT_def[:], in_=w_def_t[:])
        nc.sync.dma_start(out=b_off_sb[:], in_=b_off_t[:])
        nc.sync.dma_start(out=b_def_sb[:], in_=b_def_t[:])
        nc.sync.dma_start(out=base_sb[:], in_=base_t[:].rearrange("p t c -> p (t c)"))
        nc.sync.dma_start(out=id16[:], in_=id16_t[:])
        nc.sync.dma_start(out=id32[:], in_=id32_t[:])

        nc.vector.memset(inp_lr[:, 0:SLAB_OFF], 0.0)
        nc.vector.memset(inp_lr[:, SLAB_OFF + SLAB_ROWS * WP:], 0.0)
        nc.vector.memset(inp_hr[:, 0:SLAB_OFF], 0.0)
        nc.vector.memset(inp_hr[:, SLAB_OFF + SLAB_ROWS * WP:], 0.0)
        # split slab loads so early conv tiles start before the full load lands
        SPL = 28 * WP
        nc.sync.dma_start(
            out=inp_lr[:, SLAB_OFF:SLAB_OFF + SPL], in_=lr_slab_t[:, 0:SPL])
        nc.sync.dma_start(
            out=inp_hr[:, SLAB_OFF:SLAB_OFF + SPL], in_=hr_slab_t[:, 0:SPL])
        nc.sync.dma_start(
            out=inp_lr[:, SLAB_OFF + SPL:SLAB_OFF + SLAB_ROWS * WP],
            in_=lr_slab_t[:, SPL:])
        nc.sync.dma_start(
            out=inp_hr[:, SLAB_OFF + SPL:SLAB_OFF + SLAB_ROWS * WP],
            in_=hr_slab_t[:, SPL:])

        def _early_out():
            with tc.tile_pool(name="early_out", bufs=1) as pool:
                t = pool.tile([C, NQ], F32)
                nc.vector.memset(t[:], 0.0)
                nc.sync.dma_start(out=out_t[:], in_=t[:])

        # ---------------- offset conv (emitted in per-supertile groups) ------
        ps_off_pool = ctx.enter_context(
            tc.tile_pool(name="ps_off", bufs=1, space="PSUM"))
        ps_offT_pool = ctx.enter_context(
            tc.tile_pool(name="ps_offT", bufs=2, space="PSUM"))

        def emit_conv_tile(ot):
            u0 = SLAB_OFF + (1 + 5 * ot) * WP
            ps = ps_off_pool.tile([18, CONV_N], F32, tag="conv_ps")
            first = True
            for tap in range(NK):
                ky, kx = tap // 3, tap % 3
                shift = (ky - 1) * WP + (kx - 1)
                for c0, img in ((0, inp_lr), (1, inp_hr)):
                    nc.tensor.matmul(
                        ps[:],
                        lhsT_off[:, (c0 * NK + tap) * 18:(c0 * NK + tap + 1) * 18],
                        img[:, u0 + shift:u0 + shift + CONV_N],
                        start=first,
                        stop=(tap == NK - 1 and c0 == 1),
                    )
                    first = False
            nrow = 5 if ot < CONV_NT - 1 else 3
            src = ps[:].rearrange("p (r w) -> p r w", w=WP)[:, 0:nrow, 1:97]
            dst = off_sb[:, ot * 480: ot * 480 + nrow * 96] \
                .rearrange("p (r w) -> p r w", w=96)
            nc.vector.tensor_scalar(dst, src, b_off_sb[:], None, ALU.add)

        def emit_offT(j):
            ps = ps_offT_pool.tile([128, 128], F32, tag="offT_ps")
            nc.tensor.transpose(ps[:, 0:18],
                                off_sb[:, j * 128:(j + 1) * 128],
                                id32[0:18, 0:18])
            nc.scalar.copy(out=offT_sb[:, j * 18:(j + 1) * 18],
                           in_=ps[:, 0:18])

        # ---------------- coordinate math (batched over all tiles) ----------------
        coord = ctx.enter_context(tc.tile_pool(name="coord", bufs=1))
        s_ = coord.tile([C, NT * 18], F32, tag="s_")
        f_ = coord.tile([C, NT * 18], F32, tag="f_")
        d_ = coord.tile([C, NT * 18], F32, tag="d_")
        e_ = coord.tile([C, NT * 18], F32, tag="e_")
        idxf = coord.tile([C, NT * NK], F32, tag="idxf")
        idx16 = coord.tile([C, NT * NK], I16, tag="idx16")

        def ysel(t, sc):   # [C, STT, NK] view of y entries for supertile sc
            return t[:].rearrange("p (t k two) -> p t k two", k=NK, two=2) \
                [:, sc * STT:(sc + 1) * STT, :, 0]

        def xsel(t, sc):
            return t[:].rearrange("p (t k two) -> p t k two", k=NK, two=2) \
                [:, sc * STT:(sc + 1) * STT, :, 1]

        def ysel2(t, sc):  # same, broadcast to the rep-2 weight layout
            return t[:].rearrange("p (t k two) -> p t k two", k=NK, two=2) \
                [:, sc * STT:(sc + 1) * STT, :, 0:1] \
                .broadcast_to([C, STT, NK, 2])

        def xsel2(t, sc):
            return t[:].rearrange("p (t k two) -> p t k two", k=NK, two=2) \
                [:, sc * STT:(sc + 1) * STT, :, 1:2] \
                .broadcast_to([C, STT, NK, 2])

        def w3(t, sc):
            return t[:].rearrange("p (t k) -> p t k", k=NK) \
                [:, sc * STT:(sc + 1) * STT, :]

        def w32(t, sc):
            return t[:].rearrange("p (t k r) -> p t k r", k=NK, r=2) \
                [:, sc * STT:(sc + 1) * STT, :, :]

        # per-supertile chunks so st=0 gathers start while conv continues;
        # conv tiles are emitted just-in-time per chunk (priority order)
        SW = STQ // 16
        CONV_NEED = {0: 3, 1: 5, 2: 8, 3: 10}
        conv_done = 0
        nc.vector.memset(idx16[:], 0)
        for sc in (range(NST) if stage >= 4 else []):
            if stage >= 3:
                for ot in range(conv_done, CONV_NEED[sc]):
                    emit_conv_tile(ot)
                conv_done = CONV_NEED[sc]
            for j in range(sc * STT, (sc + 1) * STT):
                emit_offT(j)
            c2 = slice(sc * STT * 18, (sc + 1) * STT * 18)
            ck = slice(sc * STT * NK, (sc + 1) * STT * NK)
            nc.vector.tensor_add(s_[:, c2], offT_sb[:, c2], base_sb[:, c2])
            nc.vector.tensor_scalar(s_[:, c2], s_[:, c2], CLAMP_LO, CLAMP_HI,
                                    ALU.max, ALU.min)
            nc.vector.tensor_scalar(f_[:, c2], s_[:, c2], -0.5, MAGIC,
                                    ALU.add, ALU.add)
            nc.vector.tensor_scalar(f_[:, c2], f_[:, c2], MAGIC, None,
                                    ALU.subtract)
            nc.vector.tensor_sub(d_[:, c2], s_[:, c2], f_[:, c2])
            nc.vector.tensor_scalar(e_[:, c2], d_[:, c2], -1.0, 1.0,
                                    ALU.mult, ALU.add)
            nc.vector.tensor_mul(w32(w00, sc), ysel2(e_, sc), xsel2(e_, sc))
            nc.vector.tensor_mul(w32(w01, sc), ysel2(e_, sc), xsel2(d_, sc))
            nc.vector.tensor_mul(w32(w10, sc), ysel2(d_, sc), xsel2(e_, sc))
            nc.vector.tensor_mul(w32(w11, sc), ysel2(d_, sc), xsel2(d_, sc))
            nc.vector.tensor_scalar(w3(idxf, sc), ysel(f_, sc), float(WP),
                                    float(WP + 1), ALU.mult, ALU.add)
            nc.vector.tensor_add(w3(idxf, sc), w3(idxf, sc), xsel(f_, sc))
            nc.scalar.copy(out=idx16[:, ck], in_=idxf[:, ck])

            # ---- wrap for dma_gather, SBUF-only (no DRAM round trip) ----
            # idxw[b, (st, k, s)] = idx of pixel q = st*STQ + s*16 + b at
            # tap k.  q = t*128 + p with p = 16g + b and s = t9*8 + g, so
            # group g of idx16's partitions shifts to partitions 0:16
            # (DMA), then a strided DVE copy places it.
            if stage >= 5:
                stw = idxw[0:16, sc * NK * SW:(sc + 1) * NK * SW]
                idxw5 = stw.rearrange("p (k t9 g) -> p k t9 g", k=NK, t9=STT)
                for g in range(8):
                    blk = idxm[0:16, (sc * 8 + g) * STT * NK:
                               (sc * 8 + g + 1) * STT * NK]
                    nc.sync.dma_start(out=blk, in_=idx16[16 * g:16 * (g + 1), ck])
                    srcg = blk.rearrange("p (t9 k) -> p k t9", t9=STT)
                    nc.scalar.copy(out=idxw5[:, :, :, g], in_=srcg)
                # replicate to all 8 16-partition groups (log2 doubling)
                cw = slice(sc * NK * SW, (sc + 1) * NK * SW)
                nc.sync.dma_start(out=idxw[16:32, cw], in_=idxw[0:16, cw])
                nc.sync.dma_start(out=idxw[32:64, cw], in_=idxw[0:32, cw])
                nc.sync.dma_start(out=idxw[64:128, cw], in_=idxw[0:64, cw])

        # ---------------- main loop: gather / combine / transpose / einsum ----------
        gat_pool = ctx.enter_context(tc.tile_pool(name="gat", bufs=8))
        smp_pool = ctx.enter_context(tc.tile_pool(name="smp", bufs=3))
        ps_mm = ctx.enter_context(tc.tile_pool(name="ps_mm", bufs=1, space="PSUM"))
        ps_tr = ctx.enter_context(tc.tile_pool(name="ps_tr", bufs=2, space="PSUM"))
        out_pool = ctx.enter_context(tc.tile_pool(name="outp", bufs=2))

        # each gathered element: imgT2 rows [idx, idx+1] = 4C values
        #   [v00 (y0,x0) | v10 (y0+1,x0) | v01 (y0,x0+1) | v11 (y0+1,x0+1)]
        img_ap = bass.AP(tensor=imgT2_t, offset=0,
                         ap=[[2 * C, NPAD - 1], [1, 4 * C]])

        if stage < 9:
            _early_out()
        for st in (range(NST) if stage >= 6 else []):
            pse = ps_mm.tile([128, STQ], F32, tag="einsum_ps")
            for k in range(NK):
                idx_ap = idxw[:, (st * NK + k) * (STQ // 16):
                              (st * NK + k + 1) * (STQ // 16)]
                g4 = gat_pool.tile([128, STT, 4 * C], FP16, tag="g4")
                nc.gpsimd.dma_gather(g4[:], img_ap, idx_ap, STQ, STQ,
                                     elem_size=4 * C, elem_step=2 * C,
                                     single_packet=False)

                def wview(wt):
                    # [128, STT, C/2, 2]: innermost = the rep-2 pair (step 1)
                    return wt[:].rearrange("p (t k r) -> p t k r", k=NK, r=2) \
                        [:, st * STT:(st + 1) * STT, k:k + 1, :] \
                        .broadcast_to([128, STT, C // 2, 2])

                def r2(ap):
                    return ap.rearrange("p t (h r) -> p t h r", r=2)

                v00 = g4[:, :, 0:C]
                v10 = g4[:, :, C:2 * C]
                v01 = g4[:, :, 2 * C:3 * C]
                v11 = g4[:, :, 3 * C:4 * C]
                t0 = smp_pool.tile([128, STT, C], FP16, tag="t0")
                t1 = smp_pool.tile([128, STT, C], FP16, tag="t1")
                samp = smp_pool.tile([128, STT * C], FP16, tag="samp")
                samp3 = samp[:].rearrange("p (t c) -> p t c", c=C)
                if stage >= 7:
                    nc.vector.tensor_mul(r2(t0[:]), r2(v00), wview(w00))
                    nc.vector.tensor_mul(r2(t1[:]), r2(v01), wview(w01))
                    nc.vector.tensor_add(t0[:], t0[:], t1[:])
                    nc.vector.tensor_mul(r2(t1[:]), r2(v10), wview(w10))
                    nc.vector.tensor_add(t0[:], t0[:], t1[:])
                    nc.vector.tensor_mul(r2(t1[:]), r2(v11), wview(w11))
                    nc.vector.tensor_add(samp3, t0[:], t1[:])
                else:
                    nc.vector.tensor_copy(samp3, v00)

                # transposes batched 4+4+1 into PSUM, copied out in 3 ACT ops
                sampT = smp_pool.tile([128, STT * C], FP16, tag="sampT")
                for grp, glen in ((0, 4), (4, 4), (8, 1)) if stage >= 8 else []:
                    pst = ps_tr.tile([128, 512], FP16, tag="tr_ps")
                    for j2 in range(grp, grp + glen):
                        nc.tensor.transpose(pst[:, (j2 - grp) * C:
                                                 (j2 - grp + 1) * C],
                                            samp[:, j2 * C:(j2 + 1) * C],
                                            id16[:])
                    nc.scalar.copy(out=sampT[:, grp * C:(grp + glen) * C],
                                   in_=pst[:, 0:glen * C])
                for n0 in (range(0, STQ, 512) if stage >= 9 else []):
                    nn = min(512, STQ - n0)
                    nc.tensor.matmul(
                        pse[:, n0:n0 + nn],
                        lhsT_def[:, k * C:(k + 1) * C],
                        sampT[:, n0:n0 + nn],
                        start=(k == 0),
                        stop=(k == NK - 1),
                        skip_group_check=True,
                    )
            if stage >= 9:
                ost = out_pool.tile([128, STQ], F32, tag="ost")
                nc.vector.tensor_scalar(ost[:], pse[:], b_def_sb[:], None, ALU.add)
                nc.sync.dma_start(out=out_t[:, st * STQ:(st + 1) * STQ], in_=ost[:])

    nc.compile()
    if multi_queue:
        _rebalance_swdge_queues(nc)
    return nc


def _rebalance_swdge_queues(nc):
    """Align each SWDGE DMA's queue with its Tile-assigned DMASW sem lane
    (queue = lane % 4) so a given SWDGE global sem is only ever updated from
    one queue, while gathers spread across all 4 queue contexts."""
    import re
    for blk in nc.m.functions[0].blocks:
        for inst in blk.instructions:
            nm = type(inst).__name__
            is_gather = 'DMAGather' in nm
            is_pool_copy = (nm == 'InstDMACopy'
                            and str(getattr(inst, 'queue', '')).startswith('qPoolDynamic'))
            if not (is_gather or is_pool_copy):
                continue
            si = inst.sync_info
            lane = None
            if si:
                for u in si.on_update:
                    m = re.match(r'DMASW(\d+)', str(getattr(u, 'ant_name', '') or ''))
                    if m:
                        lane = int(m.group(1))
                        break
            if lane is None:
                continue
            q = lane % 4
            if is_gather:
                inst.queue_num = q
            else:
                inst.queue = 'qPoolDynamic' + ('' if q == 0 else str(q))


# ---------------- host-side prep ----------------

def _pack_weights(w_off, w_def, b_off, b_def):
    # [i, (c0, tap, o)] and [i, (k, o)] -- match SBUF lhsT layouts exactly
    w_off_p = np.ascontiguousarray(
        w_off.reshape(18, 2, C, NK).transpose(2, 1, 3, 0).reshape(C, 2 * NK * 18)
        .astype(np.float16))
    w_def_p = np.ascontiguousarray(
        w_def.reshape(C, C, NK).transpose(1, 2, 0).reshape(C, NK * C)
        .astype(np.float16))
    b_off_p = np.ascontiguousarray(b_off.reshape(18, 1).astype(np.float32))
    b_def_p = np.ascontiguousarray(b_def.reshape(C, 1).astype(np.float32))
    return w_off_p, w_def_p, b_off_p, b_def_p


def _base_grid(h):
    # base[t, tile, 2k]   = y + (k//3 - 1)   with q = h*NQ + tile*128 + t
    # base[t, tile, 2k+1] = x + (k%3 - 1)
    q = h * NQ + np.arange(NT)[None, :] * 128 + np.arange(128)[:, None]  # [128, NT]
    y = (q // W).astype(np.float32)
    x = (q % W).astype(np.float32)
    k = np.arange(NK)
    ky = (k // 3 - 1).astype(np.float32)
    kx = (k % 3 - 1).astype(np.float32)
    base = np.zeros((128, NT, 18), np.float32)
    base[:, :, 0::2] = y[:, :, None] + ky[None, None, :]
    base[:, :, 1::2] = x[:, :, None] + kx[None, None, :]
    return base


def _slab(img_b, h):
    # rows 48h-1 .. 48h+48, zero padded rows and columns, width WP, fp16
    out = np.zeros((C, SLAB_ROWS, WP), np.float16)
    r0 = HLF * h - 1
    for r in range(SLAB_ROWS):
        g = r0 + r
        if 0 <= g < H:
            out[:, r, 1:97] = img_b[:, g, :]
    return out.reshape(C, SLAB_ROWS * WP)


def _imgT2(img_b):
    # y-pair transposed padded image: row r=(y*WP+x) of the padded image,
    # imgT2[r] = [pad[:, y, x], pad[:, y+1, x]]  -> [NPAD, 2C] fp16
    pad = np.zeros((C, 101, WP), np.float16)
    pad[:, 1:97, 1:97] = img_b
    flat = np.ascontiguousarray(pad.reshape(C, 101 * WP).T)  # [9898, C]
    return np.ascontiguousarray(
        np.concatenate([flat[:NPAD], flat[WP:WP + NPAD]], axis=1))


_NC_CACHE = {}


def get_nc(num_devices=8, enable_asserts=True, stage=99, multi_queue=True):
    key = (num_devices, enable_asserts, stage, multi_queue)
    if key not in _NC_CACHE:
        _NC_CACHE[key] = _build_program(num_devices, enable_asserts,
                                        stage, multi_queue)
    return _NC_CACHE[key]


def make_in_maps(lr_features, hr_features, w_off, b_off, w_def, b_def):
    lr = np.asarray(lr_features, np.float32)
    hr = np.asarray(hr_features, np.float32)
    w_off_p, w_def_p, b_off_p, b_def_p = _pack_weights(
        np.asarray(w_off, np.float32), np.asarray(w_def, np.float32),
        np.asarray(b_off, np.float32), np.asarray(b_def, np.float32))
    id16 = np.eye(128, dtype=np.float16)
    id32 = np.eye(128, dtype=np.float32)
    bases = [_base_grid(0), _base_grid(1)]
    lr16 = lr.astype(np.float16)
    hr16 = hr.astype(np.float16)
    img2 = [_imgT2(hr16[b]) for b in range(B)]
    in_maps = []
    for core in range(8):
        b, h = core // 2, core % 2
        in_maps.append({
            "imgT2": img2[b],
            "lr_slab": _slab(lr16[b], h),
            "hr_slab": _slab(hr16[b], h),
            "w_off_p": w_off_p,
            "w_def_p": w_def_p,
            "b_off_p": b_off_p,
            "b_def_p": b_def_p,
            "base_p": bases[h],
            "ident16": id16,
            "ident32": id32,
        })
    return in_maps


def run_cores(in_maps, trace=False):
    nc = get_nc()
    res = run_bass_kernel_spmd(nc, in_maps, list(range(8)), trace=trace)
    return res


def assemble(results):
    out = np.zeros((B, C, H, W), np.float32)
    for core in range(8):
        b, h = core // 2, core % 2
        out[b, :, HLF * h:HLF * (h + 1), :] = \
            results[core]["out"].reshape(C, HLF, W)
    return out


def kernel(lr_features, hr_features, w_off, b_off, w_def, b_def):
    in_maps = make_in_maps(lr_features, hr_features, w_off, b_off, w_def, b_def)
    res = run_cores(in_maps, trace=False)
    return assemble(res.results)


# revision 37
# speedup vs baseline: 1.5624x; 1.0197x over previous
"""Trainium2 Bass kernel for DeformableConvBlock.

Reference computation:
    inp    = concat([lr, hr], axis=1)              # [B, 256, 96, 96]
    offset = conv3x3(inp, w_off, b_off)            # [B, 18, 96, 96]
    out    = deform_conv3x3(hr, offset, w_def, b_def)

Sharding: 8 cores = 4 batches x 2 half-images (48 output rows each).
Each core:
  - computes its offset conv slice (18 matmuls / tile, shifted-window APs)
  - computes sample coords/weights on DVE (pixel-partition layout,
    magic-number floor; clamp to [-1+eps, 96] + zero border reproduces
    torchvision OOB semantics exactly)
  - bilinear sampling via ONE dma_gather per tap from a host-built
    y-pair transposed padded fp16 image (each 1KB element holds all
    4 bilinear neighbors: rows (y0,x0),(y0+1,x0),(y0,x0+1),(y0+1,x0+1))
  - weighted 4-neighbor combine on DVE via tensor_scalar +
    scalar_tensor_tensor with per-partition (per-pixel) weight scalars
  - PE-transposes combined samples to channel-partition, einsum as 9
    PSUM-accumulated matmuls per 1152-pixel supertile
"""

import os
import numpy as np
from contextlib import ExitStack

import concourse.bass as bass
import concourse.tile as tile
from concourse import bacc, mybir
from concourse.bass_utils import run_bass_kernel_spmd

F32 = mybir.dt.float32
FP16 = mybir.dt.float16
I16 = mybir.dt.int16
ALU = mybir.AluOpType

# ---- geometry (hardcoded for the 4x128x96x96 problem) ----
B, C, H, W = 4, 128, 96, 96
WP = H + 2                    # 98  padded width
NPIMG = WP * WP               # 9604 padded pixels
NPAD = 76 * 128               # 9728 imgT2 rows (covers 9604 + slack)
HLF = H // 2                  # 48 output rows per core
NQ = HLF * W                  # 4608 output pixels per core
NT = NQ // 128                # 36 pixel tiles
NST = 4                       # supertiles
STT = NT // NST               # 9 tiles per supertile
STQ = STT * 128               # 1152 pixels per supertile
SLAB_ROWS = 50                # conv input rows per core (48 + halo)
SLAB_OFF = WP                 # leading zero row in conv input buffers
SLAB_ALLOC = SLAB_OFF + SLAB_ROWS * WP + 220   # slack for window overread
MAGIC = 12582912.0            # 1.5 * 2**23
CLAMP_LO = -0.9999999
CLAMP_HI = 96.0
NK = 9                        # taps
CONV_NT = 10                  # offset-conv output tiles (5 rows each)
CONV_N = 5 * WP               # 490


def _build_program(num_devices=8, enable_asserts=True, stage=99,
                   multi_queue=True):
    nc = bacc.Bacc(
        "TRN2",
        target_bir_lowering=False,
        debug=False,
        enable_asserts=enable_asserts,
        num_devices=num_devices,
        num_swdge_queues=4 if multi_queue else 1,
    )

    def inp(name, shape, dt):
        return nc.dram_tensor(name, shape, dt, kind="ExternalInput")

    imgT2_t = inp("imgT2", [NPAD, 2 * C], FP16)
    lr_slab_t = inp("lr_slab", [C, SLAB_ROWS * WP], FP16)
    hr_slab_t = inp("hr_slab", [C, SLAB_ROWS * WP], FP16)
    w_off_t = inp("w_off_p", [C, 2 * NK * 18], FP16)
    w_def_t = inp("w_def_p", [C, NK * C], FP16)
    b_off_t = inp("b_off_p", [18, 1], F32)
    b_def_t = inp("b_def_p", [C, 1], F32)
    b_defT_t = inp("b_defT_p", [1, C], FP16)
    base_t = inp("base_p", [C, NT, 18], F32)
    id16_t = inp("ident16", [128, 128], FP16)
    id32_t = inp("ident32", [128, 128], F32)

    out_t = nc.dram_tensor("out", [C, NQ], F32, kind="ExternalOutput")

    with tile.TileContext(nc) as tc, ExitStack() as ctx:
        persist = ctx.enter_context(tc.tile_pool(name="persist", bufs=1))

        # ---------------- persistent SBUF tensors ----------------
        inp_lr = persist.tile([C, SLAB_ALLOC], FP16, tag="inp_lr")
        inp_hr = persist.tile([C, SLAB_ALLOC], FP16, tag="inp_hr")
        lhsT_off = persist.tile([C, 2 * NK * 18], FP16, tag="lhsT_off")
        lhsT_def = persist.tile([C, NK * C], FP16, tag="lhsT_def")
        b_off_sb = persist.tile([18, 1], F32, tag="b_off_sb")
        b_def_sb = persist.tile([C, 1], F32, tag="b_def_sb")
        b_defT = persist.tile([1, C], FP16, tag="b_defT")
        ones_sb = persist.tile([1, STQ], FP16, tag="ones_sb")
        base_sb = persist.tile([C, NT * 18], F32, tag="base_sb")
        id16 = persist.tile([128, 128], FP16, tag="id16")
        id32 = persist.tile([128, 128], F32, tag="id32")
        off_sb = persist.tile([18, NQ], F32, tag="off_sb")
        offT_sb = persist.tile([C, NT * 18], F32, tag="offT_sb")
        # rep-2 weight planes: each weight stored twice contiguously so the
        # broadcast view's innermost dim is step-1 packed pairs (DVE 2x mode)
        w00 = persist.tile([C, NT * NK * 2], FP16, tag="w00")
        w01 = persist.tile([C, NT * NK * 2], FP16, tag="w01")
        w10 = persist.tile([C, NT * NK * 2], FP16, tag="w10")
        w11 = persist.tile([C, NT * NK * 2], FP16, tag="w11")
        idxw = persist.tile([128, NK * NST * (STQ // 16)], I16, tag="idxw")
        idxm = persist.tile([16, 8 * NT * NK], I16, tag="idxm")

        # ---------------- load constants / weights / slabs ----------------
        nc.sync.dma_start(out=lhsT_off[:], in_=w_off_t[:])
        nc.sync.dma_start(out=lhsT_def[:], in_=w_def_t[:])
        nc.sync.dma_start(out=b_off_sb[:], in_=b_off_t[:])
        nc.sync.dma_start(out=b_def_sb[:], in_=b_def_t[:])
        nc.sync.dma_start(out=b_defT[:], in_=b_defT_t[:])
        nc.vector.memset(ones_sb[:], 1.0)
        nc.sync.dma_start(out=base_sb[:], in_=base_t[:].rearrange("p t c -> p (t c)"))
        nc.sync.dma_start(out=id16[:], in_=id16_t[:])
        nc.sync.dma_start(out=id32[:], in_=id32_t[:])

        nc.vector.memset(inp_lr[:, 0:SLAB_OFF], 0.0)
        nc.vector.memset(inp_lr[:, SLAB_OFF + SLAB_ROWS * WP:], 0.0)
        nc.vector.memset(inp_hr[:, 0:SLAB_OFF], 0.0)
        nc.vector.memset(inp_hr[:, SLAB_OFF + SLAB_ROWS * WP:], 0.0)
        # split slab loads so early conv tiles start before the full load lands
        SPL = 28 * WP
        nc.sync.dma_start(
            out=inp_lr[:, SLAB_OFF:SLAB_OFF + SPL], in_=lr_slab_t[:, 0:SPL])
        nc.sync.dma_start(
            out=inp_hr[:, SLAB_OFF:SLAB_OFF + SPL], in_=hr_slab_t[:, 0:SPL])
        nc.sync.dma_start(
            out=inp_lr[:, SLAB_OFF + SPL:SLAB_OFF + SLAB_ROWS * WP],
            in_=lr_slab_t[:, SPL:])
        nc.sync.dma_start(
            out=inp_hr[:, SLAB_OFF + SPL:SLAB_OFF + SLAB_ROWS * WP],
            in_=hr_slab_t[:, SPL:])

        def _early_out():
            with tc.tile_pool(name="early_out", bufs=1) as pool:
                t = pool.tile([C, NQ], F32)
                nc.vector.memset(t[:], 0.0)
                nc.sync.dma_start(out=out_t[:], in_=t[:])

        # ---------------- offset conv (emitted in per-supertile groups) ------
        ps_off_pool = ctx.enter_context(
            tc.tile_pool(name="ps_off", bufs=1, space="PSUM"))
        ps_offT_pool = ctx.enter_context(
            tc.tile_pool(name="ps_offT", bufs=2, space="PSUM"))

        def emit_conv_tile(ot):
            u0 = SLAB_OFF + (1 + 5 * ot) * WP
            ps = ps_off_pool.tile([18, CONV_N], F32, tag="conv_ps")
            first = True
            for tap in range(NK):
                ky, kx = tap // 3, tap % 3
                shift = (ky - 1) * WP + (kx - 1)
                for c0, img in ((0, inp_lr), (1, inp_hr)):
                    nc.tensor.matmul(
                        ps[:],
                        lhsT_off[:, (c0 * NK + tap) * 18:(c0 * NK + tap + 1) * 18],
                        img[:, u0 + shift:u0 + shift + CONV_N],
                        start=first,
                        stop=(tap == NK - 1 and c0 == 1),
                    )
                    first = False
            nrow = 5 if ot < CONV_NT - 1 else 3
            src = ps[:].rearrange("p (r w) -> p r w", w=WP)[:, 0:nrow, 1:97]
            dst = off_sb[:, ot * 480: ot * 480 + nrow * 96] \
                .rearrange("p (r w) -> p r w", w=96)
            nc.vector.tensor_scalar(dst, src, b_off_sb[:], None, ALU.add)

        def emit_offT(j):
            ps = ps_offT_pool.tile([128, 128], F32, tag="offT_ps")
            nc.tensor.transpose(ps[:, 0:18],
                                off_sb[:, j * 128:(j + 1) * 128],
                                id32[0:18, 0:18])
            nc.scalar.copy(out=offT_sb[:, j * 18:(j + 1) * 18],
                           in_=ps[:, 0:18])

        # ---------------- coordinate math (batched over all tiles) ----------------
        coord = ctx.enter_context(tc.tile_pool(name="coord", bufs=1))
        s_ = coord.tile([C, NT * 18], F32, tag="s_")
        f_ = coord.tile([C, NT * 18], F32, tag="f_")
        d_ = coord.tile([C, NT * 18], F32, tag="d_")
        e_ = coord.tile([C, NT * 18], F32, tag="e_")
        idxf = coord.tile([C, NT * NK], F32, tag="idxf")
        idx16 = coord.tile([C, NT * NK], I16, tag="idx16")

        def ysel(t, sc):   # [C, STT, NK] view of y entries for supertile sc
            return t[:].rearrange("p (t k two) -> p t k two", k=NK, two=2) \
                [:, sc * STT:(sc + 1) * STT, :, 0]

        def xsel(t, sc):
            return t[:].rearrange("p (t k two) -> p t k two", k=NK, two=2) \
                [:, sc * STT:(sc + 1) * STT, :, 1]

        def ysel2(t, sc):  # same, broadcast to the rep-2 weight layout
            return t[:].rearrange("p (t k two) -> p t k two", k=NK, two=2) \
                [:, sc * STT:(sc + 1) * STT, :, 0:1] \
                .broadcast_to([C, STT, NK, 2])

        def xsel2(t, sc):
            return t[:].rearrange("p (t k two) -> p t k two", k=NK, two=2) \
                [:, sc * STT:(sc + 1) * STT, :, 1:2] \
                .broadcast_to([C, STT, NK, 2])

        def w3(t, sc):
            return t[:].rearrange("p (t k) -> p t k", k=NK) \
                [:, sc * STT:(sc + 1) * STT, :]

        def w32(t, sc):
            return t[:].rearrange("p (t k r) -> p t k r", k=NK, r=2) \
                [:, sc * STT:(sc + 1) * STT, :, :]

        # per-supertile chunks so st=0 gathers start while conv continues;
        # conv tiles are emitted just-in-time per chunk (priority order)
        SW = STQ // 16
        CONV_NEED = {0: 3, 1: 5, 2: 8, 3: 10}
        conv_done = 0
        nc.vector.memset(idx16[:], 0)
        for sc in (range(NST) if stage >= 4 else []):
            if stage >= 3:
                for ot in range(conv_done, CONV_NEED[sc]):
                    emit_conv_tile(ot)
                conv_done = CONV_NEED[sc]
            for j in range(sc * STT, (sc + 1) * STT):
                emit_offT(j)
            c2 = slice(sc * STT * 18, (sc + 1) * STT * 18)
            ck = slice(sc * STT * NK, (sc + 1) * STT * NK)
            nc.vector.tensor_add(s_[:, c2], offT_sb[:, c2], base_sb[:, c2])
            nc.vector.tensor_scalar(s_[:, c2], s_[:, c2], CLAMP_LO, CLAMP_HI,
                                    ALU.max, ALU.min)
            nc.vector.tensor_scalar(f_[:, c2], s_[:, c2], -0.5, MAGIC,
                                    ALU.add, ALU.add)
            nc.vector.tensor_scalar(f_[:, c2], f_[:, c2], MAGIC, None,
                                    ALU.subtract)
            nc.vector.tensor_sub(d_[:, c2], s_[:, c2], f_[:, c2])
            nc.vector.tensor_scalar(e_[:, c2], d_[:, c2], -1.0, 1.0,
                                    ALU.mult, ALU.add)
            nc.vector.tensor_mul(w32(w00, sc), ysel2(e_, sc), xsel2(e_, sc))
            nc.vector.tensor_mul(w32(w01, sc), ysel2(e_, sc), xsel2(d_, sc))
            nc.vector.tensor_mul(w32(w10, sc), ysel2(d_, sc), xsel2(e_, sc))
            nc.vector.tensor_mul(w32(w11, sc), ysel2(d_, sc), xsel2(d_, sc))
            nc.vector.tensor_scalar(w3(idxf, sc), ysel(f_, sc), float(WP),
                                    float(WP + 1), ALU.mult, ALU.add)
            nc.vector.tensor_add(w3(idxf, sc), w3(idxf, sc), xsel(f_, sc))
            nc.scalar.copy(out=idx16[:, ck], in_=idxf[:, ck])

            # ---- wrap for dma_gather, SBUF-only (no DRAM round trip) ----
            # idxw[b, (st, k, s)] = idx of pixel q = st*STQ + s*16 + b at
            # tap k.  q = t*128 + p with p = 16g + b and s = t9*8 + g, so
            # group g of idx16's partitions shifts to partitions 0:16
            # (DMA), then a strided DVE copy places it.
            if stage >= 5:
                stw = idxw[0:16, sc * NK * SW:(sc + 1) * NK * SW]
                idxw5 = stw.rearrange("p (k t9 g) -> p k t9 g", k=NK, t9=STT)
                for g in range(8):
                    blk = idxm[0:16, (sc * 8 + g) * STT * NK:
                               (sc * 8 + g + 1) * STT * NK]
                    nc.sync.dma_start(out=blk, in_=idx16[16 * g:16 * (g + 1), ck])
                    srcg = blk.rearrange("p (t9 k) -> p k t9", t9=STT)
                    nc.scalar.copy(out=idxw5[:, :, :, g], in_=srcg)
                # replicate to all 8 16-partition groups (log2 doubling)
                cw = slice(sc * NK * SW, (sc + 1) * NK * SW)
                nc.sync.dma_start(out=idxw[16:32, cw], in_=idxw[0:16, cw])
                nc.sync.dma_start(out=idxw[32:64, cw], in_=idxw[0:32, cw])
                nc.sync.dma_start(out=idxw[64:128, cw], in_=idxw[0:64, cw])

        # ---------------- main loop: gather / combine / transpose / einsum ----------
        gat_pool = ctx.enter_context(tc.tile_pool(name="gat", bufs=9))
        smp_pool = ctx.enter_context(tc.tile_pool(name="smp", bufs=3))
        ps_mm = ctx.enter_context(tc.tile_pool(name="ps_mm", bufs=1, space="PSUM"))
        ps_tr = ctx.enter_context(tc.tile_pool(name="ps_tr", bufs=2, space="PSUM"))
        out_pool = ctx.enter_context(tc.tile_pool(name="outp", bufs=2))

        # each gathered element: imgT2 rows [idx, idx+1] = 4C values
        #   [v00 (y0,x0) | v10 (y0+1,x0) | v01 (y0,x0+1) | v11 (y0+1,x0+1)]
        img_ap = bass.AP(tensor=imgT2_t, offset=0,
                         ap=[[2 * C, NPAD - 1], [1, 4 * C]])

        if stage < 9:
            _early_out()
        for st in (range(NST) if stage >= 6 else []):
            pse = ps_mm.tile([128, STQ], F32, tag="einsum_ps")
            for k in range(NK):
                idx_ap = idxw[:, (st * NK + k) * (STQ // 16):
                              (st * NK + k + 1) * (STQ // 16)]
                g4 = gat_pool.tile([128, STT, 4 * C], FP16, tag="g4")
                nc.gpsimd.dma_gather(g4[:], img_ap, idx_ap, STQ, STQ,
                                     elem_size=4 * C, elem_step=2 * C,
                                     single_packet=False)

                def wview(wt):
                    # [128, STT, C/2, 2]: innermost = the rep-2 pair (step 1)
                    return wt[:].rearrange("p (t k r) -> p t k r", k=NK, r=2) \
                        [:, st * STT:(st + 1) * STT, k:k + 1, :] \
                        .broadcast_to([128, STT, C // 2, 2])

                def r2(ap):
                    return ap.rearrange("p t (h r) -> p t h r", r=2)

                v00 = g4[:, :, 0:C]
                v10 = g4[:, :, C:2 * C]
                v01 = g4[:, :, 2 * C:3 * C]
                v11 = g4[:, :, 3 * C:4 * C]
                t0 = smp_pool.tile([128, STT, C], FP16, tag="t0")
                t1 = smp_pool.tile([128, STT, C], FP16, tag="t1")
                samp = smp_pool.tile([128, STT * C], FP16, tag="samp")
                samp3 = samp[:].rearrange("p (t c) -> p t c", c=C)
                if stage >= 7:
                    nc.vector.tensor_mul(r2(t0[:]), r2(v00), wview(w00))
                    nc.vector.tensor_mul(r2(t1[:]), r2(v01), wview(w01))
                    nc.vector.tensor_add(t0[:], t0[:], t1[:])
                    nc.vector.tensor_mul(r2(t1[:]), r2(v10), wview(w10))
                    nc.vector.tensor_add(t0[:], t0[:], t1[:])
                    nc.vector.tensor_mul(r2(t1[:]), r2(v11), wview(w11))
                    nc.vector.tensor_add(samp3, t0[:], t1[:])
                else:
                    nc.vector.tensor_copy(samp3, v00)

                # transposes batched 4+4+1 into PSUM, copied out in 3 ACT ops
                sampT = smp_pool.tile([128, STT * C], FP16, tag="sampT")
                for grp, glen in ((0, 4), (4, 4), (8, 1)) if stage >= 8 else []:
                    pst = ps_tr.tile([128, 512], FP16, tag="tr_ps")
                    for j2 in range(grp, grp + glen):
                        nc.tensor.transpose(pst[:, (j2 - grp) * C:
                                                 (j2 - grp + 1) * C],
                                            samp[:, j2 * C:(j2 + 1) * C],
                                            id16[:])
                    nc.scalar.copy(out=sampT[:, grp * C:(grp + glen) * C],
                                   in_=pst[:, 0:glen * C])
                for n0 in (range(0, STQ, 512) if stage >= 9 else []):
                    nn = min(512, STQ - n0)
                    nc.tensor.matmul(
                        pse[:, n0:n0 + nn],
                        lhsT_def[:, k * C:(k + 1) * C],
                        sampT[:, n0:n0 + nn],
                        start=(k == 0),
                        stop=False,
                        skip_group_check=True,
                    )
            if stage >= 9:
                for n0 in range(0, STQ, 512):
                    nn = min(512, STQ - n0)
                    nc.tensor.matmul(
                        pse[:, n0:n0 + nn], b_defT[:], ones_sb[:, n0:n0 + nn],
                        start=False, stop=True, skip_group_check=True)
                ost = out_pool.tile([128, STQ], F32, tag="ost")
                nc.scalar.copy(out=ost[:], in_=pse[:])
                nc.sync.dma_start(out=out_t[:, st * STQ:(st + 1) * STQ], in_=ost[:])

    nc.compile()
    if multi_queue:
        _rebalance_swdge_queues(nc)
    return nc


def _rebalance_swdge_queues(nc):
    """Align each SWDGE DMA's queue with its Tile-assigned DMASW sem lane
    (queue = lane % 4) so a given SWDGE global sem is only ever updated from
    one queue, while gathers spread across all 4 queue contexts."""
    import re
    for blk in nc.m.functions[0].blocks:
        for inst in blk.instructions:
            nm = type(inst).__name__
            is_gather = 'DMAGather' in nm
            is_pool_copy = (nm == 'InstDMACopy'
                            and str(getattr(inst, 'queue', '')).startswith('qPoolDynamic'))
            if not (is_gather or is_pool_copy):
                continue
            si = inst.sync_info
            lane = None
            if si:
                for u in si.on_update:
                    m = re.match(r'DMASW(\d+)', str(getattr(u, 'ant_name', '') or ''))
                    if m:
                        lane = int(m.group(1))
                        break
            if lane is None:
                continue
            q = lane % 4
            if is_gather:
                inst.queue_num = q
            else:
                inst.queue = 'qPoolDynamic' + ('' if q == 0 else str(q))


# ---------------- host-side prep ----------------

def _pack_weights(w_off, w_def, b_off, b_def):
    # [i, (c0, tap, o)] and [i, (k, o)] -- match SBUF lhsT layouts exactly
    w_off_p = np.ascontiguousarray(
        w_off.reshape(18, 2, C, NK).transpose(2, 1, 3, 0).reshape(C, 2 * NK * 18)
        .astype(np.float16))
    w_def_p = np.ascontiguousarray(
        w_def.reshape(C, C, NK).transpose(1, 2, 0).reshape(C, NK * C)
        .astype(np.float16))
    b_off_p = np.ascontiguousarray(b_off.reshape(18, 1).astype(np.float32))
    b_def_p = np.ascontiguousarray(b_def.reshape(C, 1).astype(np.float32))
    return w_off_p, w_def_p, b_off_p, b_def_p


def _base_grid(h):
    # base[t, tile, 2k]   = y + (k//3 - 1)   with q = h*NQ + tile*128 + t
    # base[t, tile, 2k+1] = x + (k%3 - 1)
    q = h * NQ + np.arange(NT)[None, :] * 128 + np.arange(128)[:, None]  # [128, NT]
    y = (q // W).astype(np.float32)
    x = (q % W).astype(np.float32)
    k = np.arange(NK)
    ky = (k // 3 - 1).astype(np.float32)
    kx = (k % 3 - 1).astype(np.float32)
    base = np.zeros((128, NT, 18), np.float32)
    base[:, :, 0::2] = y[:, :, None] + ky[None, None, :]
    base[:, :, 1::2] = x[:, :, None] + kx[None, None, :]
    return base


def _slab(img_b, h):
    # rows 48h-1 .. 48h+48, zero padded rows and columns, width WP, fp16
    out = np.zeros((C, SLAB_ROWS, WP), np.float16)
    r0 = HLF * h - 1
    for r in range(SLAB_ROWS):
        g = r0 + r
        if 0 <= g < H:
            out[:, r, 1:97] = img_b[:, g, :]
    return out.reshape(C, SLAB_ROWS * WP)


def _imgT2(img_b):
    # y-pair transposed padded image: row r=(y*WP+x) of the padded image,
    # imgT2[r] = [pad[:, y, x], pad[:, y+1, x]]  -> [NPAD, 2C] fp16
    pad = np.zeros((C, 101, WP), np.float16)
    pad[:, 1:97, 1:97] = img_b
    flat = np.ascontiguousarray(pad.reshape(C, 101 * WP).T)  # [9898, C]
    return np.ascontiguousarray(
        np.concatenate([flat[:NPAD], flat[WP:WP + NPAD]], axis=1))


_NC_CACHE = {}


def get_nc(num_devices=8, enable_asserts=True, stage=99, multi_queue=True):
    key = (num_devices, enable_asserts, stage, multi_queue)
    if key not in _NC_CACHE:
        _NC_CACHE[key] = _build_program(num_devices, enable_asserts,
                                        stage, multi_queue)
    return _NC_CACHE[key]


def make_in_maps(lr_features, hr_features, w_off, b_off, w_def, b_def):
    lr = np.asarray(lr_features, np.float32)
    hr = np.asarray(hr_features, np.float32)
    w_off_p, w_def_p, b_off_p, b_def_p = _pack_weights(
        np.asarray(w_off, np.float32), np.asarray(w_def, np.float32),
        np.asarray(b_off, np.float32), np.asarray(b_def, np.float32))
    id16 = np.eye(128, dtype=np.float16)
    id32 = np.eye(128, dtype=np.float32)
    bases = [_base_grid(0), _base_grid(1)]
    lr16 = lr.astype(np.float16)
    hr16 = hr.astype(np.float16)
    img2 = [_imgT2(hr16[b]) for b in range(B)]
    in_maps = []
    for core in range(8):
        b, h = core // 2, core % 2
        in_maps.append({
            "imgT2": img2[b],
            "lr_slab": _slab(lr16[b], h),
            "hr_slab": _slab(hr16[b], h),
            "w_off_p": w_off_p,
            "w_def_p": w_def_p,
            "b_off_p": b_off_p,
            "b_def_p": b_def_p,
            "b_defT_p": np.ascontiguousarray(
                b_def_p.reshape(1, C).astype(np.float16)),
            "base_p": bases[h],
            "ident16": id16,
            "ident32": id32,
        })
    return in_maps


def run_cores(in_maps, trace=False):
    nc = get_nc()
    res = run_bass_kernel_spmd(nc, in_maps, list(range(8)), trace=trace)
    return res


def assemble(results):
    out = np.zeros((B, C, H, W), np.float32)
    for core in range(8):
        b, h = core // 2, core % 2
        out[b, :, HLF * h:HLF * (h + 1), :] = \
            results[core]["out"].reshape(C, HLF, W)
    return out


def kernel(lr_features, hr_features, w_off, b_off, w_def, b_def):
    in_maps = make_in_maps(lr_features, hr_features, w_off, b_off, w_def, b_def)
    res = run_cores(in_maps, trace=False)
    return assemble(res.results)
